# revision 20
# baseline (speedup 1.0000x reference)
"""Multi-head causal attention (B=4, S=2048, D=1024, H=16) on 8 TRN2 cores.

Sharding: core c -> batch c//2, head-group c%2 (8 heads, 512 of the 1024
QKV columns / Wo rows).  Each core runs a fused QKV->attention->out-proj
kernel on its shard; the host sums the two head-group partials per batch.

Per-core layout choices:
  - x is fed pre-transposed (xT [D, S]) so Q^T/K^T come out of the PE in
    [m, s] layout and V in natural [s, m] layout with no on-chip transposes.
  - scores are computed transposed (S^T [k, q]); softmax runs as
    exp (ScalarE, scale=1/8 fused) -> causal mask (gpsimd affine_select,
    fill=0, diagonal tiles only, masked q-ranges skipped entirely) ->
    attnV matmul with a ones-column appended to V (M=65) so the softmax
    denominator accumulates for free in PSUM row 64.
  - normalization: DVE reciprocal of row 64, gpsimd partition_broadcast,
    one DVE multiply into C^T [m, s].
  - out-proj emits out^T [n, s]; the host transposes back.
All matmul inputs are bf16 (1 cycle/row on the PE; fp32r is a 2-pass
format at ~2 cycles/row); accumulation stays fp32 in PSUM.
"""

import numpy as np

B, S, D = 4, 2048, 1024
H, DH = 16, 64
HPC = 8            # heads per core
M = HPC * DH       # 512: per-core qkv out dim / wo in dim
NCORE = 8
CH = 512           # q/s chunk size
NCH = S // CH      # 4
ND = D // 128      # 8  d-tiles (contraction for qkv proj)
NMT = M // 128     # 4  m-tiles (= head pairs)
NKT = S // 128     # 16 k-tiles
NNT = D // 128     # 8  n-tiles (out proj)

LAST_RESULT = None  # BassKernelResults of the most recent run (for test.py)


def _emit(nc, tc, tile, mybir, aps):
    import concourse.bass as bass  # noqa: F401

    f32 = mybir.dt.float32
    bf16 = mybir.dt.bfloat16
    EXP = mybir.ActivationFunctionType.Exp
    xT, wq, wk, wv, wo, ones8, outT = aps

    def r(ap):
        return ap

    with (
        tc.tile_pool(name="w", bufs=1) as pw,
        tc.tile_pool(name="kv", bufs=1) as pkv,
        tc.tile_pool(name="qt", bufs=2) as pq,
        tc.tile_pool(name="ct", bufs=1) as pct,
        tc.tile_pool(name="x", bufs=1) as px,
        tc.tile_pool(name="u", bufs=4) as pu,
        tc.tile_pool(name="sm", bufs=2) as psm,
        tc.tile_pool(name="o", bufs=2) as po,
        tc.tile_pool(name="cn", bufs=2) as pcn,
        tc.tile_pool(name="dscratch", bufs=2, space="DRAM") as pdram,
        tc.tile_pool(name="ps_mm", bufs=2, space="PSUM") as pp_mm,
        tc.tile_pool(name="ps_sc", bufs=1, space="PSUM") as pp_sc,
        tc.tile_pool(name="ps_av", bufs=4, space="PSUM") as pp_av,
    ):
        # ---- weights ----
        wq_sb, wk_sb, wv_sb = [], [], []
        for d in range(ND):
            for lst, src, nm in ((wq_sb, wq, "wq"), (wk_sb, wk, "wk"), (wv_sb, wv, "wv")):
                t = pw.tile([128, M], bf16, name=f"{nm}{d}", tag=f"{nm}{d}")
                nc.sync.dma_start(out=t, in_=src[128 * d:128 * (d + 1), :])
                lst.append(t)
        wo_sb = []
        for t in range(NMT):
            w = pw.tile([128, D], bf16, name=f"wo{t}", tag=f"wo{t}")
            nc.sync.dma_start(out=w, in_=wo[128 * t:128 * (t + 1), :])
            wo_sb.append(w)

        # ---- V storage: [s, 8 heads x (64 V + 1 ones)] ----
        vau = []
        for st in range(NKT):
            v = pkv.tile([128, HPC * 65], bf16, name=f"vau{st}", tag=f"vau{st}")
            nc.sync.dma_start(
                out=v.rearrange("p (h c) -> p h c", c=65)[:, :, 64:65],
                in_=ones8.rearrange("p (h c) -> p h c", c=1),
            )
            vau.append(v)
        kt_sb = [[None] * NCH for _ in range(NMT)]

        for j in range(NCH):  # ---- chunk loop ----
            # x^T chunk [d, s]
            xt = []
            for d in range(ND):
                x_t = px.tile([128, CH], bf16, name=f"x{d}", tag=f"x{d}")
                nc.sync.dma_start(
                    out=x_t, in_=xT[128 * d:128 * (d + 1), CH * j:CH * (j + 1)]
                )
                xt.append(x_t)

            # Q^T, K^T projections -> [m, s]
            qt = []
            for t in range(NMT):
                ps = pp_mm.tile([128, CH], f32, name="psq", tag="mm")
                for d in range(ND):
                    nc.tensor.matmul(
                        ps,
                        lhsT=r(wq_sb[d][:, 128 * t:128 * (t + 1)]),
                        rhs=r(xt[d]),
                        start=(d == 0),
                        stop=(d == ND - 1),
                    )
                q_t = pq.tile([128, CH], bf16, name=f"q{t}", tag=f"q{t}")
                nc.vector.tensor_copy(out=q_t, in_=ps)
                qt.append(q_t)
            for t in range(NMT):
                ps = pp_mm.tile([128, CH], f32, name="psk", tag="mm")
                for d in range(ND):
                    nc.tensor.matmul(
                        ps,
                        lhsT=r(wk_sb[d][:, 128 * t:128 * (t + 1)]),
                        rhs=r(xt[d]),
                        start=(d == 0),
                        stop=(d == ND - 1),
                    )
                k_t = pkv.tile([128, CH], bf16, name=f"k{t}_{j}", tag=f"k{t}_{j}")
                nc.vector.tensor_copy(out=k_t, in_=ps)
                kt_sb[t][j] = k_t
            # V projection -> natural [s, m], strided into vau (65-col groups)
            for st in range(CH // 128):
                ps = pp_mm.tile([128, M], f32, name="psv", tag="mm")
                for d in range(ND):
                    nc.tensor.matmul(
                        ps,
                        lhsT=r(xt[d][:, 128 * st:128 * (st + 1)]),
                        rhs=r(wv_sb[d]),
                        start=(d == 0),
                        stop=(d == ND - 1),
                    )
                g = vau[4 * j + st]
                nc.vector.tensor_copy(
                    out=g.rearrange("p (h c) -> p h c", c=65)[:, :, 0:64],
                    in_=ps.rearrange("p (h c) -> p h c", c=64),
                )

            # ---- attention, one head-pair at a time ----
            ct = []
            cn = []
            den8 = psm.tile([HPC, CH], f32, name="den8", tag="den8")
            for t in range(NMT):
                av = [
                    pp_av.tile([65, CH], f32, name=f"av{h}", tag="av")
                    for h in range(2)
                ]
                nkt = 4 * (j + 1)
                for kt in range(nkt):
                    dd = kt - 4 * j          # diagonal index (>=0 on diag)
                    qoff = 128 * dd if dd >= 0 else 0
                    n = CH - qoff
                    ck, ks = kt // 4, (kt % 4) * 128
                    # both heads' scores in one 2-bank PSUM tile
                    sc = pp_sc.tile([128, 2 * CH], f32, name="sc", tag="sc")
                    for h in range(2):
                        pb = 64 * h
                        nc.tensor.matmul(
                            sc[:, CH * h:CH * h + n],
                            lhsT=r(kt_sb[t][ck][pb:pb + 64, ks:ks + 128]),
                            rhs=r(qt[t][pb:pb + 64, qoff:CH]),
                            start=True,
                            stop=True,
                            tile_position=(pb, 0),
                        )
                    u = pu.tile([128, 2 * CH], bf16, name="u", tag="u")
                    scv = sc.rearrange("p (h q) -> p h q", h=2)[:, :, 0:n]
                    uv = u.rearrange("p (h q) -> p h q", h=2)[:, :, 0:n]
                    nc.scalar.activation(out=uv, in_=scv, func=EXP, scale=0.125)
                    if dd >= 0:
                        # keep where q_rel >= k_partition (same mask both heads)
                        nc.gpsimd.affine_select(
                            out=uv,
                            in_=uv,
                            compare_op=mybir.AluOpType.is_ge,
                            fill=0.0,
                            base=0,
                            channel_multiplier=-1,
                            pattern=[[0, 2], [1, n]],
                        )
                    for h in range(2):
                        ha = 2 * t + h
                        nc.tensor.matmul(
                            av[h][:, qoff:CH],
                            lhsT=r(vau[kt][:, 65 * ha:65 * ha + 65]),
                            rhs=r(u[:, CH * h:CH * h + n]),
                            start=(kt == 0),
                            stop=(kt == nkt - 1),
                        )
                # drain PSUM quickly: unnormalized C (bf16) + denom rows (f32)
                cn_t = pcn.tile([128, CH], bf16, name=f"cn{t}", tag=f"cn{t}")
                for h in range(2):
                    nc.vector.tensor_copy(
                        out=cn_t[64 * h:64 * (h + 1), :], in_=av[h][0:64, :]
                    )
                    dstage = psm.tile([1, CH], f32, name="dstage", tag="dstage", bufs=4)
                    nc.vector.tensor_copy(out=dstage, in_=av[h][64:65, :])
                    nc.sync.dma_start(
                        out=den8[2 * t + h:2 * t + h + 1, :], in_=dstage
                    )
                cn.append(cn_t)

            # one batched reciprocal for all 8 heads, then normalize
            rec8 = psm.tile([HPC, CH], f32, name="rec8", tag="rec8")
            nc.vector.reciprocal(out=rec8, in_=den8)
            rec_d = pdram.tile([HPC, CH], f32, name="recd", tag="recd")
            nc.sync.dma_start(out=rec_d, in_=rec8)
            for t in range(NMT):
                c_t = pct.tile([128, CH], bf16, name=f"c{t}", tag=f"c{t}")
                bc = psm.tile([128, CH], f32, name="bc", tag="bc", bufs=4)
                for h in range(2):
                    nc.sync.dma_start(
                        out=bc[64 * h:64 * (h + 1), :],
                        in_=rec_d[2 * t + h:2 * t + h + 1, :].to_broadcast((64, CH)),
                    )
                for h in range(2):
                    nc.vector.tensor_mul(
                        c_t[64 * h:64 * (h + 1), :],
                        cn[t][64 * h:64 * (h + 1), :],
                        bc[64 * h:64 * (h + 1), :],
                    )
                ct.append(c_t)

            # ---- out projection (transposed): out^T[n, s] ----
            for nt in range(NNT):
                ps = pp_mm.tile([128, CH], f32, name="pso", tag="mm")
                for t in range(NMT):
                    nc.tensor.matmul(
                        ps,
                        lhsT=r(wo_sb[t][:, 128 * nt:128 * (nt + 1)]),
                        rhs=r(ct[t]),
                        start=(t == 0),
                        stop=(t == NMT - 1),
                    )
                o_sb = po.tile([128, CH], f32, name="osb", tag="o")
                nc.vector.tensor_copy(out=o_sb, in_=ps)
                nc.sync.dma_start(
                    out=outT[128 * nt:128 * (nt + 1), CH * j:CH * (j + 1)],
                    in_=o_sb,
                )


_PROG = None


def _build():
    global _PROG
    if _PROG is not None:
        return _PROG
    import concourse.bacc as bacc
    import concourse.mybir as mybir
    import concourse.tile as tile

    f32 = mybir.dt.float32
    bf16 = mybir.dt.bfloat16
    nc = bacc.Bacc(
        "TRN2", target_bir_lowering=False, debug=False, enable_asserts=False
    )
    xT = nc.dram_tensor("xT", [D, S], bf16, kind="ExternalInput").ap()
    wq = nc.dram_tensor("wq", [D, M], bf16, kind="ExternalInput").ap()
    wk = nc.dram_tensor("wk", [D, M], bf16, kind="ExternalInput").ap()
    wv = nc.dram_tensor("wv", [D, M], bf16, kind="ExternalInput").ap()
    wo = nc.dram_tensor("wo", [M, D], bf16, kind="ExternalInput").ap()
    ones8 = nc.dram_tensor("ones8", [128, HPC], bf16, kind="ExternalInput").ap()
    outT = nc.dram_tensor("outT", [D, S], f32, kind="ExternalOutput").ap()

    with tile.TileContext(nc) as tc:
        _emit(nc, tc, tile, mybir, (xT, wq, wk, wv, wo, ones8, outT))
    nc.compile()
    _PROG = nc
    return nc


def kernel(x, Wq, Wk, Wv, Wo, bo):
    global LAST_RESULT
    import os

    from concourse.bass_utils import run_bass_kernel_spmd

    x = np.asarray(x, dtype=np.float32)
    Wq = np.asarray(Wq, dtype=np.float32)
    Wk = np.asarray(Wk, dtype=np.float32)
    Wv = np.asarray(Wv, dtype=np.float32)
    Wo = np.asarray(Wo, dtype=np.float32)
    bo = np.asarray(bo, dtype=np.float32)

    nc = _build()

    import ml_dtypes

    bf = ml_dtypes.bfloat16
    in_maps = []
    for c in range(NCORE):
        b, g = c // 2, c % 2
        cols = slice(M * g, M * (g + 1))
        in_maps.append(
            {
                "xT": np.ascontiguousarray(x[b].T).astype(bf),
                "wq": np.ascontiguousarray(Wq[:, cols]).astype(bf),
                "wk": np.ascontiguousarray(Wk[:, cols]).astype(bf),
                "wv": np.ascontiguousarray(Wv[:, cols]).astype(bf),
                "wo": np.ascontiguousarray(Wo[cols, :]).astype(bf),
                "ones8": np.ones((128, HPC), dtype=bf),
            }
        )

    res = run_bass_kernel_spmd(
        nc,
        in_maps,
        list(range(NCORE)),
        trace=bool(os.environ.get("KERNEL_TRACE")),
        tmpdir=os.environ.get("KERNEL_TRACE_DIR") or None,
    )
    LAST_RESULT = res

    out = np.empty((B, S, D), dtype=np.float32)
    for b in range(B):
        acc = res.results[2 * b]["outT"] + res.results[2 * b + 1]["outT"]
        out[b] = acc.T + bo[None, :]
    return out


# revision 22
# speedup vs baseline: 1.3198x; 1.3198x over previous
"""Multi-head causal attention (B=4, S=2048, D=1024, H=16) on 8 TRN2 cores.

Sharding: core c -> batch c//2, head-group c%2 (8 heads, 512 of the 1024
QKV columns / Wo rows).  Each core runs a fused QKV->attention->out-proj
kernel on its shard; the host sums the two head-group partials per batch.

Per-core layout choices:
  - x is fed pre-transposed (xT [D, S]) so Q^T/K^T come out of the PE in
    [m, s] layout and V in natural [s, m] layout with no on-chip transposes.
  - scores are computed transposed (S^T [k, q]); softmax runs as
    exp (ScalarE, scale=1/8 fused) -> causal mask (gpsimd affine_select,
    fill=0, diagonal tiles only, masked q-ranges skipped entirely) ->
    attnV matmul with a ones-column appended to V (M=65) so the softmax
    denominator accumulates for free in PSUM row 64.
  - normalization: DVE reciprocal of row 64, gpsimd partition_broadcast,
    one DVE multiply into C^T [m, s].
  - out-proj emits out^T [n, s]; the host transposes back.
All matmul inputs are bf16 (1 cycle/row on the PE; fp32r is a 2-pass
format at ~2 cycles/row); accumulation stays fp32 in PSUM.
"""

import numpy as np

B, S, D = 4, 2048, 1024
H, DH = 16, 64
HPC = 8            # heads per core
M = HPC * DH       # 512: per-core qkv out dim / wo in dim
NCORE = 8
CH = 512           # q/s chunk size
NCH = S // CH      # 4
ND = D // 128      # 8  d-tiles (contraction for qkv proj)
NMT = M // 128     # 4  m-tiles (= head pairs)
NKT = S // 128     # 16 k-tiles
NNT = D // 128     # 8  n-tiles (out proj)

LAST_RESULT = None  # BassKernelResults of the most recent run (for test.py)


def _emit(nc, tc, tile, mybir, aps):
    import concourse.bass as bass  # noqa: F401

    f32 = mybir.dt.float32
    bf16 = mybir.dt.bfloat16
    EXP = mybir.ActivationFunctionType.Exp
    xT, wq, wk, wv, wo, ones8, outT = aps

    def r(ap):
        return ap

    with (
        tc.tile_pool(name="w", bufs=1) as pw,
        tc.tile_pool(name="kv", bufs=1) as pkv,
        tc.tile_pool(name="qt", bufs=2) as pq,
        tc.tile_pool(name="ct", bufs=1) as pct,
        tc.tile_pool(name="x", bufs=2) as px,
        tc.tile_pool(name="u", bufs=4) as pu,
        tc.tile_pool(name="sm", bufs=2) as psm,
        tc.tile_pool(name="o", bufs=2) as po,
        tc.tile_pool(name="cn", bufs=2) as pcn,
        tc.tile_pool(name="dscratch", bufs=2, space="DRAM") as pdram,
        tc.tile_pool(name="ps_mm", bufs=2, space="PSUM") as pp_mm,
        tc.tile_pool(name="ps_out", bufs=1, space="PSUM") as pp_out,
        tc.tile_pool(name="ps_sc", bufs=1, space="PSUM") as pp_sc,
        tc.tile_pool(name="ps_av", bufs=3, space="PSUM") as pp_av,
    ):
        # ---- weights ----
        wq_sb, wk_sb, wv_sb = [], [], []
        for d in range(ND):
            for lst, src, nm in ((wq_sb, wq, "wq"), (wk_sb, wk, "wk"), (wv_sb, wv, "wv")):
                t = pw.tile([128, M], bf16, name=f"{nm}{d}", tag=f"{nm}{d}")
                nc.sync.dma_start(out=t, in_=src[128 * d:128 * (d + 1), :])
                lst.append(t)
        wo_sb = []
        for t in range(NMT):
            w = pw.tile([128, D], bf16, name=f"wo{t}", tag=f"wo{t}")
            nc.sync.dma_start(out=w, in_=wo[128 * t:128 * (t + 1), :])
            wo_sb.append(w)

        # ---- V storage: [s, 8 heads x (64 V + 1 ones)] ----
        vau = []
        for st in range(NKT):
            v = pkv.tile([128, HPC * 65], bf16, name=f"vau{st}", tag=f"vau{st}")
            nc.sync.dma_start(
                out=v.rearrange("p (h c) -> p h c", c=65)[:, :, 64:65],
                in_=ones8.rearrange("p (h c) -> p h c", c=1),
            )
            vau.append(v)
        kt_sb = [[None] * NCH for _ in range(NMT)]

        for j in range(NCH):  # ---- chunk loop ----
            # x^T chunk [d, s]
            xt = []
            for d in range(ND):
                x_t = px.tile([128, CH], bf16, name=f"x{d}", tag=f"x{d}")
                nc.sync.dma_start(
                    out=x_t, in_=xT[128 * d:128 * (d + 1), CH * j:CH * (j + 1)]
                )
                xt.append(x_t)

            # Q^T, K^T projections -> [m, s]
            qt = []
            for t in range(NMT):
                ps = pp_mm.tile([128, CH], f32, name="psq", tag="mm")
                for d in range(ND):
                    nc.tensor.matmul(
                        ps,
                        lhsT=r(wq_sb[d][:, 128 * t:128 * (t + 1)]),
                        rhs=r(xt[d]),
                        start=(d == 0),
                        stop=(d == ND - 1),
                    )
                q_t = pq.tile([128, CH], bf16, name=f"q{t}", tag=f"q{t}")
                nc.vector.tensor_copy(out=q_t, in_=ps)
                qt.append(q_t)
            for t in range(NMT):
                ps = pp_mm.tile([128, CH], f32, name="psk", tag="mm")
                for d in range(ND):
                    nc.tensor.matmul(
                        ps,
                        lhsT=r(wk_sb[d][:, 128 * t:128 * (t + 1)]),
                        rhs=r(xt[d]),
                        start=(d == 0),
                        stop=(d == ND - 1),
                    )
                k_t = pkv.tile([128, CH], bf16, name=f"k{t}_{j}", tag=f"k{t}_{j}")
                nc.vector.tensor_copy(out=k_t, in_=ps)
                kt_sb[t][j] = k_t
            # V projection -> natural [s, m], strided into vau (65-col groups)
            for st in range(CH // 128):
                ps = pp_mm.tile([128, M], f32, name="psv", tag="mm")
                for d in range(ND):
                    nc.tensor.matmul(
                        ps,
                        lhsT=r(xt[d][:, 128 * st:128 * (st + 1)]),
                        rhs=r(wv_sb[d]),
                        start=(d == 0),
                        stop=(d == ND - 1),
                    )
                g = vau[4 * j + st]
                nc.vector.tensor_copy(
                    out=g.rearrange("p (h c) -> p h c", c=65)[:, :, 0:64],
                    in_=ps.rearrange("p (h c) -> p h c", c=64),
                )

            # ---- attention, one head-pair at a time ----
            ct = []
            cn = []
            den8 = psm.tile([HPC, CH], f32, name="den8", tag="den8")
            for t in range(NMT):
                av = [
                    pp_av.tile([65, CH], f32, name=f"av{h}", tag="av")
                    for h in range(2)
                ]
                nkt = 4 * (j + 1)
                for kt in range(nkt):
                    dd = kt - 4 * j          # diagonal index (>=0 on diag)
                    qoff = 128 * dd if dd >= 0 else 0
                    n = CH - qoff
                    ck, ks = kt // 4, (kt % 4) * 128
                    # both heads' scores in one 2-bank PSUM tile
                    sc = pp_sc.tile([128, 2 * CH], f32, name="sc", tag="sc")
                    for h in range(2):
                        pb = 64 * h
                        nc.tensor.matmul(
                            sc[:, CH * h:CH * h + n],
                            lhsT=r(kt_sb[t][ck][pb:pb + 64, ks:ks + 128]),
                            rhs=r(qt[t][pb:pb + 64, qoff:CH]),
                            start=True,
                            stop=True,
                            tile_position=(pb, 0),
                        )
                    u = pu.tile([128, 2 * CH], bf16, name="u", tag="u")
                    scv = sc.rearrange("p (h q) -> p h q", h=2)[:, :, 0:n]
                    uv = u.rearrange("p (h q) -> p h q", h=2)[:, :, 0:n]
                    nc.scalar.activation(out=uv, in_=scv, func=EXP, scale=0.125)
                    if dd >= 0:
                        # keep where q_rel >= k_partition (same mask both heads)
                        nc.gpsimd.affine_select(
                            out=uv,
                            in_=uv,
                            compare_op=mybir.AluOpType.is_ge,
                            fill=0.0,
                            base=0,
                            channel_multiplier=-1,
                            pattern=[[0, 2], [1, n]],
                        )
                    for h in range(2):
                        ha = 2 * t + h
                        nc.tensor.matmul(
                            av[h][:, qoff:CH],
                            lhsT=r(vau[kt][:, 65 * ha:65 * ha + 65]),
                            rhs=r(u[:, CH * h:CH * h + n]),
                            start=(kt == 0),
                            stop=(kt == nkt - 1),
                        )
                # drain PSUM quickly: unnormalized C (bf16) + denom rows (f32)
                cn_t = pcn.tile([128, CH], bf16, name=f"cn{t}", tag=f"cn{t}")
                for h in range(2):
                    nc.vector.tensor_copy(
                        out=cn_t[64 * h:64 * (h + 1), :], in_=av[h][0:64, :]
                    )
                    dstage = psm.tile([1, CH], f32, name="dstage", tag="dstage", bufs=4)
                    nc.vector.tensor_copy(out=dstage, in_=av[h][64:65, :])
                    nc.sync.dma_start(
                        out=den8[2 * t + h:2 * t + h + 1, :], in_=dstage
                    )
                cn.append(cn_t)

            # one batched reciprocal for all 8 heads, then normalize
            rec8 = psm.tile([HPC, CH], f32, name="rec8", tag="rec8")
            nc.vector.reciprocal(out=rec8, in_=den8)
            rec_d = pdram.tile([HPC, CH], f32, name="recd", tag="recd")
            nc.sync.dma_start(out=rec_d, in_=rec8)
            for t in range(NMT):
                c_t = pct.tile([128, CH], bf16, name=f"c{t}", tag=f"c{t}")
                bc = psm.tile([128, CH], f32, name="bc", tag="bc", bufs=4)
                for h in range(2):
                    nc.sync.dma_start(
                        out=bc[64 * h:64 * (h + 1), :],
                        in_=rec_d[2 * t + h:2 * t + h + 1, :].to_broadcast((64, CH)),
                    )
                for h in range(2):
                    nc.vector.tensor_mul(
                        c_t[64 * h:64 * (h + 1), :],
                        cn[t][64 * h:64 * (h + 1), :],
                        bc[64 * h:64 * (h + 1), :],
                    )
                ct.append(c_t)

            # ---- out projection (transposed): out^T[n, s] ----
            for nt in range(NNT):
                ps = pp_out.tile([128, CH], f32, name="pso", tag="out")
                for t in range(NMT):
                    nc.tensor.matmul(
                        ps,
                        lhsT=r(wo_sb[t][:, 128 * nt:128 * (nt + 1)]),
                        rhs=r(ct[t]),
                        start=(t == 0),
                        stop=(t == NMT - 1),
                    )
                o_sb = po.tile([128, CH], f32, name="osb", tag="o")
                nc.vector.tensor_copy(out=o_sb, in_=ps)
                nc.sync.dma_start(
                    out=outT[128 * nt:128 * (nt + 1), CH * j:CH * (j + 1)],
                    in_=o_sb,
                )


_PROG = None


def _build():
    global _PROG
    if _PROG is not None:
        return _PROG
    import concourse.bacc as bacc
    import concourse.mybir as mybir
    import concourse.tile as tile

    f32 = mybir.dt.float32
    bf16 = mybir.dt.bfloat16
    nc = bacc.Bacc(
        "TRN2", target_bir_lowering=False, debug=False, enable_asserts=False
    )
    xT = nc.dram_tensor("xT", [D, S], bf16, kind="ExternalInput").ap()
    wq = nc.dram_tensor("wq", [D, M], bf16, kind="ExternalInput").ap()
    wk = nc.dram_tensor("wk", [D, M], bf16, kind="ExternalInput").ap()
    wv = nc.dram_tensor("wv", [D, M], bf16, kind="ExternalInput").ap()
    wo = nc.dram_tensor("wo", [M, D], bf16, kind="ExternalInput").ap()
    ones8 = nc.dram_tensor("ones8", [128, HPC], bf16, kind="ExternalInput").ap()
    outT = nc.dram_tensor("outT", [D, S], f32, kind="ExternalOutput").ap()

    with tile.TileContext(nc) as tc:
        _emit(nc, tc, tile, mybir, (xT, wq, wk, wv, wo, ones8, outT))
    nc.compile()
    _PROG = nc
    return nc


def kernel(x, Wq, Wk, Wv, Wo, bo):
    global LAST_RESULT
    import os

    from concourse.bass_utils import run_bass_kernel_spmd

    x = np.asarray(x, dtype=np.float32)
    Wq = np.asarray(Wq, dtype=np.float32)
    Wk = np.asarray(Wk, dtype=np.float32)
    Wv = np.asarray(Wv, dtype=np.float32)
    Wo = np.asarray(Wo, dtype=np.float32)
    bo = np.asarray(bo, dtype=np.float32)

    nc = _build()

    import ml_dtypes

    bf = ml_dtypes.bfloat16
    in_maps = []
    for c in range(NCORE):
        b, g = c // 2, c % 2
        cols = slice(M * g, M * (g + 1))
        in_maps.append(
            {
                "xT": np.ascontiguousarray(x[b].T).astype(bf),
                "wq": np.ascontiguousarray(Wq[:, cols]).astype(bf),
                "wk": np.ascontiguousarray(Wk[:, cols]).astype(bf),
                "wv": np.ascontiguousarray(Wv[:, cols]).astype(bf),
                "wo": np.ascontiguousarray(Wo[cols, :]).astype(bf),
                "ones8": np.ones((128, HPC), dtype=bf),
            }
        )

    res = run_bass_kernel_spmd(
        nc,
        in_maps,
        list(range(NCORE)),
        trace=bool(os.environ.get("KERNEL_TRACE")),
        tmpdir=os.environ.get("KERNEL_TRACE_DIR") or None,
    )
    LAST_RESULT = res

    out = np.empty((B, S, D), dtype=np.float32)
    for b in range(B):
        acc = res.results[2 * b]["outT"] + res.results[2 * b + 1]["outT"]
        out[b] = acc.T + bo[None, :]
    return out


# revision 24
# speedup vs baseline: 1.3621x; 1.0320x over previous
"""Multi-head causal attention (B=4, S=2048, D=1024, H=16) on 8 TRN2 cores.

Sharding: core c -> batch c//2, head-group c%2 (8 heads, 512 of the 1024
QKV columns / Wo rows).  Each core runs a fused QKV->attention->out-proj
kernel on its shard; the host sums the two head-group partials per batch.

Per-core layout choices:
  - x is fed pre-transposed (xT [D, S]) so Q^T/K^T come out of the PE in
    [m, s] layout and V in natural [s, m] layout with no on-chip transposes.
  - scores are computed transposed (S^T [k, q]); softmax runs as
    exp (ScalarE, scale=1/8 fused, both heads of a pair in one op) ->
    causal mask (gpsimd affine_select, fill=0, diagonal tiles only,
    fully-masked q-ranges skipped entirely) -> attnV matmul with a
    ones-column appended to V (M=65) so the softmax denominator
    accumulates for free in PSUM row 64.
  - normalization: denominator rows are gathered into one [8, S-chunk]
    tile (via SBUF->SBUF DMA; compute engines cannot address partition
    bases other than 0/32/64/96), one batched DVE reciprocal, then a
    DRAM-bounced broadcast DMA and one DVE multiply into C^T [m, s].
  - out-proj emits out^T [n, s]; the host transposes back.
  - Tile builds STATIC per-engine instruction streams, so next-chunk QKV
    and previous-chunk out-proj matmuls are explicitly interleaved into
    the attention kt-loop to keep the PE dense (and HAM un-throttled)
    while ScalarE works on exp.
All matmul inputs are bf16 (1 cycle/row on the PE; fp32r is a 2-pass
format at ~2 cycles/row); accumulation stays fp32 in PSUM.
"""

import numpy as np

B, S, D = 4, 2048, 1024
H, DH = 16, 64
HPC = 8            # heads per core
M = HPC * DH       # 512: per-core qkv out dim / wo in dim
NCORE = 8
CH = 512           # q/s chunk size
NCH = S // CH      # 4
ND = D // 128      # 8  d-tiles (contraction for qkv proj)
NMT = M // 128     # 4  m-tiles (= head pairs)
NKT = S // 128     # 16 k-tiles
NNT = D // 128     # 8  n-tiles (out proj)

LAST_RESULT = None  # BassKernelResults of the most recent run (for test.py)


def _emit(nc, tc, tile, mybir, aps):
    import concourse.bass as bass  # noqa: F401

    f32 = mybir.dt.float32
    bf16 = mybir.dt.bfloat16
    EXP = mybir.ActivationFunctionType.Exp
    xT, wq, wk, wv, wo, ones8, outT = aps

    with (
        tc.tile_pool(name="w", bufs=1) as pw,
        tc.tile_pool(name="kv", bufs=1) as pkv,
        tc.tile_pool(name="qt", bufs=2) as pq,
        tc.tile_pool(name="ct", bufs=2) as pct,
        tc.tile_pool(name="x", bufs=2) as px,
        tc.tile_pool(name="u", bufs=4) as pu,
        tc.tile_pool(name="sm", bufs=2) as psm,
        tc.tile_pool(name="o", bufs=2) as po,
        tc.tile_pool(name="cn", bufs=2) as pcn,
        tc.tile_pool(name="dscratch", bufs=2, space="DRAM") as pdram,
        tc.tile_pool(name="ps_mm", bufs=2, space="PSUM") as pp_mm,
        tc.tile_pool(name="ps_out", bufs=1, space="PSUM") as pp_out,
        tc.tile_pool(name="ps_sc", bufs=1, space="PSUM") as pp_sc,
        tc.tile_pool(name="ps_av", bufs=3, space="PSUM") as pp_av,
    ):
        # ---- weights ----
        wq_sb, wk_sb, wv_sb = [], [], []
        for d in range(ND):
            for lst, src, nm in (
                (wq_sb, wq, "wq"), (wk_sb, wk, "wk"), (wv_sb, wv, "wv")
            ):
                t = pw.tile([128, M], bf16, name=f"{nm}{d}", tag=f"{nm}{d}")
                nc.sync.dma_start(out=t, in_=src[128 * d:128 * (d + 1), :])
                lst.append(t)
        wo_sb = []
        for t in range(NMT):
            w = pw.tile([128, D], bf16, name=f"wo{t}", tag=f"wo{t}")
            nc.sync.dma_start(out=w, in_=wo[128 * t:128 * (t + 1), :])
            wo_sb.append(w)

        # ---- V storage: [s, 8 heads x (64 V + 1 ones)] ----
        vau = []
        for st in range(NKT):
            v = pkv.tile([128, HPC * 65], bf16, name=f"vau{st}", tag=f"vau{st}")
            nc.sync.dma_start(
                out=v.rearrange("p (h c) -> p h c", c=65)[:, :, 64:65],
                in_=ones8.rearrange("p (h c) -> p h c", c=1),
            )
            vau.append(v)
        kt_sb = [[None] * NCH for _ in range(NMT)]
        qt_all = {}   # j -> [4 tiles]
        ct_all = {}   # j -> [4 tiles]

        # ---------- emission units ----------
        def x_load(j):
            xt = []
            for d in range(ND):
                x_t = px.tile([128, CH], bf16, name=f"x{d}", tag=f"x{d}")
                nc.sync.dma_start(
                    out=x_t, in_=xT[128 * d:128 * (d + 1), CH * j:CH * (j + 1)]
                )
                xt.append(x_t)
            return xt

        def proj_half(ps, w_sb, t, xt, half, kind):
            """4 of the 8 contraction steps of one projection m-tile."""
            for d in range(4 * half, 4 * half + 4):
                if kind == "v":
                    lhsT = xt[d][:, 128 * t:128 * (t + 1)]
                    rhs = w_sb[d]
                else:
                    lhsT = w_sb[d][:, 128 * t:128 * (t + 1)]
                    rhs = xt[d]
                nc.tensor.matmul(
                    ps, lhsT=lhsT, rhs=rhs,
                    start=(d == 0), stop=(d == ND - 1),
                )

        def qkv_units(j):
            """Generator of emission closures for chunk j's QKV projection."""
            xt = []

            def do_xload():
                xt.extend(x_load(j))
            yield do_xload

            qts = []
            qt_all[j] = qts
            for t in range(NMT):
                ps_box = []

                def qa(t=t, ps_box=ps_box):
                    ps = pp_mm.tile([128, CH], f32, name="psq", tag="mm")
                    ps_box.append(ps)
                    proj_half(ps, wq_sb, t, xt, 0, "q")
                def qb(t=t, ps_box=ps_box):
                    ps = ps_box[0]
                    proj_half(ps, wq_sb, t, xt, 1, "q")
                    q_t = pq.tile([128, CH], bf16, name=f"q{t}", tag=f"q{t}")
                    nc.vector.tensor_copy(out=q_t, in_=ps)
                    qts.append(q_t)
                yield qa
                yield qb
            for t in range(NMT):
                ps_box = []

                def ka(t=t, ps_box=ps_box):
                    ps = pp_mm.tile([128, CH], f32, name="psk", tag="mm")
                    ps_box.append(ps)
                    proj_half(ps, wk_sb, t, xt, 0, "k")
                def kb(t=t, ps_box=ps_box, j=j):
                    ps = ps_box[0]
                    proj_half(ps, wk_sb, t, xt, 1, "k")
                    k_t = pkv.tile(
                        [128, CH], bf16, name=f"k{t}_{j}", tag=f"k{t}_{j}"
                    )
                    nc.vector.tensor_copy(out=k_t, in_=ps)
                    kt_sb[t][j] = k_t
                yield ka
                yield kb
            for st in range(NMT):
                ps_box = []

                def va(st=st, ps_box=ps_box):
                    ps = pp_mm.tile([128, M], f32, name="psv", tag="mm")
                    ps_box.append(ps)
                    proj_half(ps, wv_sb, st, xt, 0, "v")
                def vb(st=st, ps_box=ps_box, j=j):
                    ps = ps_box[0]
                    proj_half(ps, wv_sb, st, xt, 1, "v")
                    g = vau[4 * j + st]
                    nc.vector.tensor_copy(
                        out=g.rearrange("p (h c) -> p h c", c=65)[:, :, 0:64],
                        in_=ps.rearrange("p (h c) -> p h c", c=64),
                    )
                yield va
                yield vb

        def outproj_units(j):
            """Generator of emission closures for chunk j's out-projection."""
            for nt in range(NNT):
                def og(nt=nt, j=j):
                    ct = ct_all[j]
                    ps = pp_out.tile([128, CH], f32, name="pso", tag="out")
                    for t in range(NMT):
                        nc.tensor.matmul(
                            ps,
                            lhsT=wo_sb[t][:, 128 * nt:128 * (nt + 1)],
                            rhs=ct[t],
                            start=(t == 0),
                            stop=(t == NMT - 1),
                        )
                    o_sb = po.tile([128, CH], f32, name="osb", tag="o")
                    nc.vector.tensor_copy(out=o_sb, in_=ps)
                    nc.sync.dma_start(
                        out=outT[128 * nt:128 * (nt + 1), CH * j:CH * (j + 1)],
                        in_=o_sb,
                    )
                yield og

        # ---------- chunk 0 QKV up front ----------
        for unit in qkv_units(0):
            unit()

        # ---------- main loop: attention(j) with interleaved fillers ----------
        for j in range(NCH):
            fillers = []
            if j + 1 < NCH:
                fillers.extend(qkv_units(j + 1))
            if j >= 1:
                fillers.extend(outproj_units(j - 1))
            nkt = 4 * (j + 1)
            n_units = NMT * nkt
            stride = max(1, n_units // max(1, len(fillers)))
            ucount = 0

            qt = qt_all[j]
            ct = []
            ct_all[j] = ct
            den8 = psm.tile([HPC, CH], f32, name="den8", tag="den8")
            for t in range(NMT):
                av = [
                    pp_av.tile([65, CH], f32, name=f"av{h}", tag="av")
                    for h in range(2)
                ]
                for kt in range(nkt):
                    dd = kt - 4 * j          # diagonal index (>=0 on diag)
                    qoff = 128 * dd if dd >= 0 else 0
                    n = CH - qoff
                    ck, ks = kt // 4, (kt % 4) * 128
                    # both heads' scores in one 2-bank PSUM tile
                    sc = pp_sc.tile([128, 2 * CH], f32, name="sc", tag="sc")
                    for h in range(2):
                        pb = 64 * h
                        nc.tensor.matmul(
                            sc[:, CH * h:CH * h + n],
                            lhsT=kt_sb[t][ck][pb:pb + 64, ks:ks + 128],
                            rhs=qt[t][pb:pb + 64, qoff:CH],
                            start=True,
                            stop=True,
                            tile_position=(pb, 0),
                        )
                    u = pu.tile([128, 2 * CH], bf16, name="u", tag="u")
                    scv = sc.rearrange("p (h q) -> p h q", h=2)[:, :, 0:n]
                    uv = u.rearrange("p (h q) -> p h q", h=2)[:, :, 0:n]
                    nc.scalar.activation(out=uv, in_=scv, func=EXP, scale=0.125)
                    if dd >= 0:
                        # keep where q_rel >= k_partition (same mask, both heads)
                        nc.gpsimd.affine_select(
                            out=uv,
                            in_=uv,
                            compare_op=mybir.AluOpType.is_ge,
                            fill=0.0,
                            base=0,
                            channel_multiplier=-1,
                            pattern=[[0, 2], [1, n]],
                        )
                    for h in range(2):
                        ha = 2 * t + h
                        nc.tensor.matmul(
                            av[h][:, qoff:CH],
                            lhsT=vau[kt][:, 65 * ha:65 * ha + 65],
                            rhs=u[:, CH * h:CH * h + n],
                            start=(kt == 0),
                            stop=(kt == nkt - 1),
                        )
                    ucount += 1
                    if fillers and ucount % stride == 0:
                        fillers.pop(0)()

                # drain PSUM quickly: unnormalized C (bf16) + denom rows (f32)
                cn_t = pcn.tile([128, CH], bf16, name=f"cn{t}", tag=f"cn{t}")
                for h in range(2):
                    nc.vector.tensor_copy(
                        out=cn_t[64 * h:64 * (h + 1), :], in_=av[h][0:64, :]
                    )
                    dstage = psm.tile(
                        [1, CH], f32, name="dstage", tag="dstage", bufs=4
                    )
                    nc.vector.tensor_copy(out=dstage, in_=av[h][64:65, :])
                    nc.sync.dma_start(
                        out=den8[2 * t + h:2 * t + h + 1, :], in_=dstage
                    )
                ct.append(cn_t)  # placeholder; replaced after normalize

            # one batched reciprocal for all 8 heads, then normalize
            rec8 = psm.tile([HPC, CH], f32, name="rec8", tag="rec8")
            nc.vector.reciprocal(out=rec8, in_=den8)
            rec_d = pdram.tile([HPC, CH], f32, name="recd", tag="recd")
            nc.sync.dma_start(out=rec_d, in_=rec8)
            cn = list(ct)
            for t in range(NMT):
                c_t = pct.tile([128, CH], bf16, name=f"c{t}", tag=f"c{t}")
                bc = psm.tile([128, CH], f32, name="bc", tag="bc", bufs=4)
                for h in range(2):
                    nc.sync.dma_start(
                        out=bc[64 * h:64 * (h + 1), :],
                        in_=rec_d[2 * t + h:2 * t + h + 1, :].to_broadcast(
                            (64, CH)
                        ),
                    )
                for h in range(2):
                    nc.vector.tensor_mul(
                        c_t[64 * h:64 * (h + 1), :],
                        cn[t][64 * h:64 * (h + 1), :],
                        bc[64 * h:64 * (h + 1), :],
                    )
                ct[t] = c_t

            # leftover fillers for this round
            for f in fillers:
                f()

        # final chunk's out-projection
        for unit in outproj_units(NCH - 1):
            unit()


_PROG = None


def _build():
    global _PROG
    if _PROG is not None:
        return _PROG
    import concourse.bacc as bacc
    import concourse.mybir as mybir
    import concourse.tile as tile

    f32 = mybir.dt.float32
    bf16 = mybir.dt.bfloat16
    nc = bacc.Bacc(
        "TRN2", target_bir_lowering=False, debug=False, enable_asserts=False
    )
    xT = nc.dram_tensor("xT", [D, S], bf16, kind="ExternalInput").ap()
    wq = nc.dram_tensor("wq", [D, M], bf16, kind="ExternalInput").ap()
    wk = nc.dram_tensor("wk", [D, M], bf16, kind="ExternalInput").ap()
    wv = nc.dram_tensor("wv", [D, M], bf16, kind="ExternalInput").ap()
    wo = nc.dram_tensor("wo", [M, D], bf16, kind="ExternalInput").ap()
    ones8 = nc.dram_tensor("ones8", [128, HPC], bf16, kind="ExternalInput").ap()
    outT = nc.dram_tensor("outT", [D, S], f32, kind="ExternalOutput").ap()

    with tile.TileContext(nc) as tc:
        _emit(nc, tc, tile, mybir, (xT, wq, wk, wv, wo, ones8, outT))
    nc.compile()
    _PROG = nc
    return nc


def kernel(x, Wq, Wk, Wv, Wo, bo):
    global LAST_RESULT
    import os

    from concourse.bass_utils import run_bass_kernel_spmd

    x = np.asarray(x, dtype=np.float32)
    Wq = np.asarray(Wq, dtype=np.float32)
    Wk = np.asarray(Wk, dtype=np.float32)
    Wv = np.asarray(Wv, dtype=np.float32)
    Wo = np.asarray(Wo, dtype=np.float32)
    bo = np.asarray(bo, dtype=np.float32)

    nc = _build()

    import ml_dtypes

    bf = ml_dtypes.bfloat16
    in_maps = []
    for c in range(NCORE):
        b, g = c // 2, c % 2
        cols = slice(M * g, M * (g + 1))
        in_maps.append(
            {
                "xT": np.ascontiguousarray(x[b].T).astype(bf),
                "wq": np.ascontiguousarray(Wq[:, cols]).astype(bf),
                "wk": np.ascontiguousarray(Wk[:, cols]).astype(bf),
                "wv": np.ascontiguousarray(Wv[:, cols]).astype(bf),
                "wo": np.ascontiguousarray(Wo[cols, :]).astype(bf),
                "ones8": np.ones((128, HPC), dtype=bf),
            }
        )

    res = run_bass_kernel_spmd(
        nc,
        in_maps,
        list(range(NCORE)),
        trace=bool(os.environ.get("KERNEL_TRACE")),
        tmpdir=os.environ.get("KERNEL_TRACE_DIR") or None,
    )
    LAST_RESULT = res

    out = np.empty((B, S, D), dtype=np.float32)
    for b in range(B):
        acc = res.results[2 * b]["outT"] + res.results[2 * b + 1]["outT"]
        out[b] = acc.T + bo[None, :]
    return out


# revision 25
# speedup vs baseline: 1.7873x; 1.3122x over previous
"""Multi-head causal attention (B=4, S=2048, D=1024, H=16) on 8 TRN2 cores.

Sharding: core c -> batch c//2, head-group c%2 (8 heads, 512 of the 1024
QKV columns / Wo rows).  Each core runs a fused QKV->attention->out-proj
kernel on its shard; the host sums the two head-group partials per batch.

Per-core layout choices:
  - x is fed pre-transposed (xT [D, S]) so Q^T/K^T come out of the PE in
    [m, s] layout and V in natural [s, m] layout with no on-chip transposes.
  - scores are computed transposed (S^T [k, q]); softmax runs as
    exp (ScalarE, scale=1/8 fused, both heads of a pair in one op) ->
    causal mask (gpsimd affine_select, fill=0, diagonal tiles only,
    fully-masked q-ranges skipped entirely) -> attnV matmul with a
    ones-column appended to V (M=65) so the softmax denominator
    accumulates for free in PSUM row 64.
  - normalization: denominator rows are gathered into one [8, S-chunk]
    tile (via SBUF->SBUF DMA; compute engines cannot address partition
    bases other than 0/32/64/96), one batched DVE reciprocal, then a
    DRAM-bounced broadcast DMA and one DVE multiply into C^T [m, s].
  - out-proj emits out^T [n, s]; the host transposes back.
  - Tile builds STATIC per-engine instruction streams, so next-chunk QKV
    and previous-chunk out-proj matmuls are explicitly interleaved into
    the attention kt-loop to keep the PE dense (and HAM un-throttled)
    while ScalarE works on exp.
All matmul inputs are bf16 (1 cycle/row on the PE; fp32r is a 2-pass
format at ~2 cycles/row); accumulation stays fp32 in PSUM.
"""

import numpy as np

B, S, D = 4, 2048, 1024
H, DH = 16, 64
HPC = 8            # heads per core
M = HPC * DH       # 512: per-core qkv out dim / wo in dim
NCORE = 8
CH = 512           # q/s chunk size
NCH = S // CH      # 4
ND = D // 128      # 8  d-tiles (contraction for qkv proj)
NMT = M // 128     # 4  m-tiles (= head pairs)
NKT = S // 128     # 16 k-tiles
NNT = D // 128     # 8  n-tiles (out proj)

LAST_RESULT = None  # BassKernelResults of the most recent run (for test.py)


def _emit(nc, tc, tile, mybir, aps):
    import concourse.bass as bass  # noqa: F401

    f32 = mybir.dt.float32
    bf16 = mybir.dt.bfloat16
    EXP = mybir.ActivationFunctionType.Exp
    xT, wq, wk, wv, wo, ones8, outT = aps

    with (
        tc.tile_pool(name="w", bufs=1) as pw,
        tc.tile_pool(name="kv", bufs=1) as pkv,
        tc.tile_pool(name="qt", bufs=2) as pq,
        tc.tile_pool(name="ct", bufs=2) as pct,
        tc.tile_pool(name="x", bufs=2) as px,
        tc.tile_pool(name="u", bufs=4) as pu,
        tc.tile_pool(name="sm", bufs=2) as psm,
        tc.tile_pool(name="o", bufs=2) as po,
        tc.tile_pool(name="cn", bufs=2) as pcn,
        tc.tile_pool(name="dscratch", bufs=2, space="DRAM") as pdram,
        tc.tile_pool(name="ps_mm", bufs=2, space="PSUM") as pp_mm,
        tc.tile_pool(name="ps_sc", bufs=2, space="PSUM") as pp_sc,
        tc.tile_pool(name="ps_av", bufs=2, space="PSUM") as pp_av,
    ):
        # ---- weights ----
        wq_sb, wk_sb, wv_sb = [], [], []
        for d in range(ND):
            for lst, src, nm in (
                (wq_sb, wq, "wq"), (wk_sb, wk, "wk"), (wv_sb, wv, "wv")
            ):
                t = pw.tile([128, M], bf16, name=f"{nm}{d}", tag=f"{nm}{d}")
                nc.sync.dma_start(out=t, in_=src[128 * d:128 * (d + 1), :])
                lst.append(t)
        wo_sb = []
        for t in range(NMT):
            w = pw.tile([128, D], bf16, name=f"wo{t}", tag=f"wo{t}")
            nc.sync.dma_start(out=w, in_=wo[128 * t:128 * (t + 1), :])
            wo_sb.append(w)

        # ---- V storage: [s, 8 heads x (64 V + 1 ones)] ----
        vau = []
        for st in range(NKT):
            v = pkv.tile([128, HPC * 65], bf16, name=f"vau{st}", tag=f"vau{st}")
            nc.sync.dma_start(
                out=v.rearrange("p (h c) -> p h c", c=65)[:, :, 64:65],
                in_=ones8.rearrange("p (h c) -> p h c", c=1),
            )
            vau.append(v)
        kt_sb = [[None] * NCH for _ in range(NMT)]
        qt_all = {}   # j -> [4 tiles]
        ct_all = {}   # j -> [4 tiles]

        # ---------- emission units ----------
        def x_load(j):
            xt = []
            for d in range(ND):
                x_t = px.tile([128, CH], bf16, name=f"x{d}", tag=f"x{d}")
                nc.sync.dma_start(
                    out=x_t, in_=xT[128 * d:128 * (d + 1), CH * j:CH * (j + 1)]
                )
                xt.append(x_t)
            return xt

        def proj_half(ps, w_sb, t, xt, half, kind):
            """4 of the 8 contraction steps of one projection m-tile."""
            for d in range(4 * half, 4 * half + 4):
                if kind == "v":
                    lhsT = xt[d][:, 128 * t:128 * (t + 1)]
                    rhs = w_sb[d]
                else:
                    lhsT = w_sb[d][:, 128 * t:128 * (t + 1)]
                    rhs = xt[d]
                nc.tensor.matmul(
                    ps, lhsT=lhsT, rhs=rhs,
                    start=(d == 0), stop=(d == ND - 1),
                )

        def qkv_units(j):
            """Generator of emission closures for chunk j's QKV projection."""
            xt = []

            def do_xload():
                xt.extend(x_load(j))
            yield do_xload

            qts = []
            qt_all[j] = qts
            for t in range(NMT):
                ps_box = []

                def qa(t=t, ps_box=ps_box):
                    ps = pp_mm.tile([128, CH], f32, name="psq", tag="mm")
                    ps_box.append(ps)
                    proj_half(ps, wq_sb, t, xt, 0, "q")
                def qb(t=t, ps_box=ps_box):
                    ps = ps_box[0]
                    proj_half(ps, wq_sb, t, xt, 1, "q")
                    q_t = pq.tile([128, CH], bf16, name=f"q{t}", tag=f"q{t}")
                    nc.vector.tensor_copy(out=q_t, in_=ps)
                    qts.append(q_t)
                yield qa
                yield qb
            for t in range(NMT):
                ps_box = []

                def ka(t=t, ps_box=ps_box):
                    ps = pp_mm.tile([128, CH], f32, name="psk", tag="mm")
                    ps_box.append(ps)
                    proj_half(ps, wk_sb, t, xt, 0, "k")
                def kb(t=t, ps_box=ps_box, j=j):
                    ps = ps_box[0]
                    proj_half(ps, wk_sb, t, xt, 1, "k")
                    k_t = pkv.tile(
                        [128, CH], bf16, name=f"k{t}_{j}", tag=f"k{t}_{j}"
                    )
                    nc.vector.tensor_copy(out=k_t, in_=ps)
                    kt_sb[t][j] = k_t
                yield ka
                yield kb
            for st in range(NMT):
                ps_box = []

                def va(st=st, ps_box=ps_box):
                    ps = pp_mm.tile([128, M], f32, name="psv", tag="mm")
                    ps_box.append(ps)
                    proj_half(ps, wv_sb, st, xt, 0, "v")
                def vb(st=st, ps_box=ps_box, j=j):
                    ps = ps_box[0]
                    proj_half(ps, wv_sb, st, xt, 1, "v")
                    g = vau[4 * j + st]
                    nc.vector.tensor_copy(
                        out=g.rearrange("p (h c) -> p h c", c=65)[:, :, 0:64],
                        in_=ps.rearrange("p (h c) -> p h c", c=64),
                    )
                yield va
                yield vb

        def outproj_units(j):
            """Generator of emission closures for chunk j's out-projection."""
            for nt in range(NNT):
                def og(nt=nt, j=j):
                    ct = ct_all[j]
                    ps = pp_mm.tile([128, CH], f32, name="pso", tag="mm")
                    for t in range(NMT):
                        nc.tensor.matmul(
                            ps,
                            lhsT=wo_sb[t][:, 128 * nt:128 * (nt + 1)],
                            rhs=ct[t],
                            start=(t == 0),
                            stop=(t == NMT - 1),
                        )
                    o_sb = po.tile([128, CH], f32, name="osb", tag="o")
                    nc.vector.tensor_copy(out=o_sb, in_=ps)
                    nc.sync.dma_start(
                        out=outT[128 * nt:128 * (nt + 1), CH * j:CH * (j + 1)],
                        in_=o_sb,
                    )
                yield og

        # ---------- chunk 0 QKV up front ----------
        for unit in qkv_units(0):
            unit()

        # ---------- main loop: attention(j) with interleaved fillers ----------
        for j in range(NCH):
            fillers = []
            if j + 1 < NCH:
                fillers.extend(qkv_units(j + 1))
            if j >= 1:
                fillers.extend(outproj_units(j - 1))
            nkt = 4 * (j + 1)
            n_units = NMT * nkt
            stride = max(1, n_units // max(1, len(fillers)))
            ucount = 0

            qt = qt_all[j]
            ct = []
            ct_all[j] = ct
            den8 = psm.tile([HPC, CH], f32, name="den8", tag="den8")
            for t in range(NMT):
                av = [
                    pp_av.tile([65, CH], f32, name=f"av{h}", tag="av")
                    for h in range(2)
                ]
                us = {}
                for kt in range(nkt + 1):
                    if kt < nkt:
                        dd = kt - 4 * j      # diagonal index (>=0 on diag)
                        qoff = 128 * dd if dd >= 0 else 0
                        n = CH - qoff
                        ck, ks = kt // 4, (kt % 4) * 128
                        # both heads' scores in one 2-bank PSUM tile
                        sc = pp_sc.tile([128, 2 * CH], f32, name="sc", tag="sc")
                        for h in range(2):
                            pb = 64 * h
                            nc.tensor.matmul(
                                sc[:, CH * h:CH * h + n],
                                lhsT=kt_sb[t][ck][pb:pb + 64, ks:ks + 128],
                                rhs=qt[t][pb:pb + 64, qoff:CH],
                                start=True,
                                stop=True,
                                tile_position=(pb, 0),
                            )
                        u = pu.tile([128, 2 * CH], bf16, name="u", tag="u")
                        scv = sc.rearrange("p (h q) -> p h q", h=2)[:, :, 0:n]
                        uv = u.rearrange("p (h q) -> p h q", h=2)[:, :, 0:n]
                        nc.scalar.activation(out=uv, in_=scv, func=EXP, scale=0.125)
                        if dd >= 0:
                            # keep where q_rel >= k_partition (same mask, both)
                            nc.gpsimd.affine_select(
                                out=uv,
                                in_=uv,
                                compare_op=mybir.AluOpType.is_ge,
                                fill=0.0,
                                base=0,
                                channel_multiplier=-1,
                                pattern=[[0, 2], [1, n]],
                            )
                        us[kt] = (u, qoff, n)
                    if kt >= 1:
                        # attnV for the PREVIOUS kt (exp latency hidden by
                        # the scores matmul above and the filler below)
                        pkt = kt - 1
                        u_p, qoff_p, n_p = us.pop(pkt)
                        for h in range(2):
                            ha = 2 * t + h
                            nc.tensor.matmul(
                                av[h][:, qoff_p:CH],
                                lhsT=vau[pkt][:, 65 * ha:65 * ha + 65],
                                rhs=u_p[:, CH * h:CH * h + n_p],
                                start=(pkt == 0),
                                stop=(pkt == nkt - 1),
                            )
                    ucount += 1
                    if fillers and ucount % stride == 0:
                        fillers.pop(0)()

                # drain PSUM quickly: unnormalized C (bf16) + denom rows (f32)
                cn_t = pcn.tile([128, CH], bf16, name=f"cn{t}", tag=f"cn{t}")
                for h in range(2):
                    nc.vector.tensor_copy(
                        out=cn_t[64 * h:64 * (h + 1), :], in_=av[h][0:64, :]
                    )
                    dstage = psm.tile(
                        [1, CH], f32, name="dstage", tag="dstage", bufs=4
                    )
                    nc.vector.tensor_copy(out=dstage, in_=av[h][64:65, :])
                    nc.sync.dma_start(
                        out=den8[2 * t + h:2 * t + h + 1, :], in_=dstage
                    )
                ct.append(cn_t)  # placeholder; replaced after normalize

            # one batched reciprocal for all 8 heads, then normalize
            rec8 = psm.tile([HPC, CH], f32, name="rec8", tag="rec8")
            nc.vector.reciprocal(out=rec8, in_=den8)
            rec_d = pdram.tile([HPC, CH], f32, name="recd", tag="recd")
            nc.sync.dma_start(out=rec_d, in_=rec8)
            cn = list(ct)
            for t in range(NMT):
                c_t = pct.tile([128, CH], bf16, name=f"c{t}", tag=f"c{t}")
                bc = psm.tile([128, CH], f32, name="bc", tag="bc", bufs=4)
                for h in range(2):
                    nc.sync.dma_start(
                        out=bc[64 * h:64 * (h + 1), :],
                        in_=rec_d[2 * t + h:2 * t + h + 1, :].to_broadcast(
                            (64, CH)
                        ),
                    )
                for h in range(2):
                    nc.vector.tensor_mul(
                        c_t[64 * h:64 * (h + 1), :],
                        cn[t][64 * h:64 * (h + 1), :],
                        bc[64 * h:64 * (h + 1), :],
                    )
                ct[t] = c_t

            # leftover fillers for this round
            for f in fillers:
                f()

        # final chunk's out-projection
        for unit in outproj_units(NCH - 1):
            unit()


_PROG = None


def _build():
    global _PROG
    if _PROG is not None:
        return _PROG
    import concourse.bacc as bacc
    import concourse.mybir as mybir
    import concourse.tile as tile

    f32 = mybir.dt.float32
    bf16 = mybir.dt.bfloat16
    nc = bacc.Bacc(
        "TRN2", target_bir_lowering=False, debug=False, enable_asserts=False
    )
    xT = nc.dram_tensor("xT", [D, S], bf16, kind="ExternalInput").ap()
    wq = nc.dram_tensor("wq", [D, M], bf16, kind="ExternalInput").ap()
    wk = nc.dram_tensor("wk", [D, M], bf16, kind="ExternalInput").ap()
    wv = nc.dram_tensor("wv", [D, M], bf16, kind="ExternalInput").ap()
    wo = nc.dram_tensor("wo", [M, D], bf16, kind="ExternalInput").ap()
    ones8 = nc.dram_tensor("ones8", [128, HPC], bf16, kind="ExternalInput").ap()
    outT = nc.dram_tensor("outT", [D, S], f32, kind="ExternalOutput").ap()

    with tile.TileContext(nc) as tc:
        _emit(nc, tc, tile, mybir, (xT, wq, wk, wv, wo, ones8, outT))
    nc.compile()
    _PROG = nc
    return nc


def kernel(x, Wq, Wk, Wv, Wo, bo):
    global LAST_RESULT
    import os

    from concourse.bass_utils import run_bass_kernel_spmd

    x = np.asarray(x, dtype=np.float32)
    Wq = np.asarray(Wq, dtype=np.float32)
    Wk = np.asarray(Wk, dtype=np.float32)
    Wv = np.asarray(Wv, dtype=np.float32)
    Wo = np.asarray(Wo, dtype=np.float32)
    bo = np.asarray(bo, dtype=np.float32)

    nc = _build()

    import ml_dtypes

    bf = ml_dtypes.bfloat16
    in_maps = []
    for c in range(NCORE):
        b, g = c // 2, c % 2
        cols = slice(M * g, M * (g + 1))
        in_maps.append(
            {
                "xT": np.ascontiguousarray(x[b].T).astype(bf),
                "wq": np.ascontiguousarray(Wq[:, cols]).astype(bf),
                "wk": np.ascontiguousarray(Wk[:, cols]).astype(bf),
                "wv": np.ascontiguousarray(Wv[:, cols]).astype(bf),
                "wo": np.ascontiguousarray(Wo[cols, :]).astype(bf),
                "ones8": np.ones((128, HPC), dtype=bf),
            }
        )

    res = run_bass_kernel_spmd(
        nc,
        in_maps,
        list(range(NCORE)),
        trace=bool(os.environ.get("KERNEL_TRACE")),
        tmpdir=os.environ.get("KERNEL_TRACE_DIR") or None,
    )
    LAST_RESULT = res

    out = np.empty((B, S, D), dtype=np.float32)
    for b in range(B):
        acc = res.results[2 * b]["outT"] + res.results[2 * b + 1]["outT"]
        out[b] = acc.T + bo[None, :]
    return out


# revision 26
# speedup vs baseline: 1.8299x; 1.0238x over previous
"""Multi-head causal attention (B=4, S=2048, D=1024, H=16) on 8 TRN2 cores.

Sharding: core c -> batch c//2, head-group c%2 (8 heads, 512 of the 1024
QKV columns / Wo rows).  Each core runs a fused QKV->attention->out-proj
kernel on its shard; the host sums the two head-group partials per batch.

Per-core layout choices:
  - x is fed pre-transposed (xT [D, S]) so Q^T/K^T come out of the PE in
    [m, s] layout and V in natural [s, m] layout with no on-chip transposes.
  - scores are computed transposed (S^T [k, q]); softmax runs as
    exp (ScalarE, scale=1/8 fused, both heads of a pair in one op) ->
    causal mask (gpsimd affine_select, fill=0, diagonal tiles only,
    fully-masked q-ranges skipped entirely) -> attnV matmul with a
    ones-column appended to V (M=65) so the softmax denominator
    accumulates for free in PSUM row 64.
  - normalization: denominator rows are gathered into one [8, S-chunk]
    tile (via SBUF->SBUF DMA; compute engines cannot address partition
    bases other than 0/32/64/96), one batched DVE reciprocal, then a
    DRAM-bounced broadcast DMA and one DVE multiply into C^T [m, s].
  - out-proj emits out^T [n, s]; the host transposes back.
  - Tile builds STATIC per-engine instruction streams, so next-chunk QKV
    and previous-chunk out-proj matmuls are explicitly interleaved into
    the attention kt-loop to keep the PE dense (and HAM un-throttled)
    while ScalarE works on exp.
All matmul inputs are bf16 (1 cycle/row on the PE; fp32r is a 2-pass
format at ~2 cycles/row); accumulation stays fp32 in PSUM.
"""

import numpy as np

B, S, D = 4, 2048, 1024
H, DH = 16, 64
HPC = 8            # heads per core
M = HPC * DH       # 512: per-core qkv out dim / wo in dim
NCORE = 8
CH = 512           # q/s chunk size
NCH = S // CH      # 4
ND = D // 128      # 8  d-tiles (contraction for qkv proj)
NMT = M // 128     # 4  m-tiles (= head pairs)
NKT = S // 128     # 16 k-tiles
NNT = D // 128     # 8  n-tiles (out proj)

LAST_RESULT = None  # BassKernelResults of the most recent run (for test.py)


def _emit(nc, tc, tile, mybir, aps):
    import concourse.bass as bass  # noqa: F401

    f32 = mybir.dt.float32
    bf16 = mybir.dt.bfloat16
    EXP = mybir.ActivationFunctionType.Exp
    xT, wq, wk, wv, wo, ones8, outT = aps

    with (
        tc.tile_pool(name="w", bufs=1) as pw,
        tc.tile_pool(name="kv", bufs=1) as pkv,
        tc.tile_pool(name="qt", bufs=2) as pq,
        tc.tile_pool(name="ct", bufs=2) as pct,
        tc.tile_pool(name="x", bufs=2) as px,
        tc.tile_pool(name="u", bufs=6) as pu,
        tc.tile_pool(name="sm", bufs=2) as psm,
        tc.tile_pool(name="o", bufs=2) as po,
        tc.tile_pool(name="cn", bufs=2) as pcn,
        tc.tile_pool(name="dscratch", bufs=2, space="DRAM") as pdram,
        tc.tile_pool(name="ps_mm", bufs=2, space="PSUM") as pp_mm,
        tc.tile_pool(name="ps_sc", bufs=2, space="PSUM") as pp_sc,
        tc.tile_pool(name="ps_av", bufs=2, space="PSUM") as pp_av,
    ):
        # ---- weights ----
        wq_sb, wk_sb, wv_sb = [], [], []
        for d in range(ND):
            for lst, src, nm in (
                (wq_sb, wq, "wq"), (wk_sb, wk, "wk"), (wv_sb, wv, "wv")
            ):
                t = pw.tile([128, M], bf16, name=f"{nm}{d}", tag=f"{nm}{d}")
                nc.sync.dma_start(out=t, in_=src[128 * d:128 * (d + 1), :])
                lst.append(t)
        wo_sb = []
        for t in range(NMT):
            w = pw.tile([128, D], bf16, name=f"wo{t}", tag=f"wo{t}")
            nc.sync.dma_start(out=w, in_=wo[128 * t:128 * (t + 1), :])
            wo_sb.append(w)

        # ---- V storage: [s, 8 heads x (64 V + 1 ones)] ----
        vau = []
        for st in range(NKT):
            v = pkv.tile([128, HPC * 65], bf16, name=f"vau{st}", tag=f"vau{st}")
            nc.sync.dma_start(
                out=v.rearrange("p (h c) -> p h c", c=65)[:, :, 64:65],
                in_=ones8.rearrange("p (h c) -> p h c", c=1),
            )
            vau.append(v)
        kt_sb = [[None] * NCH for _ in range(NMT)]
        qt_all = {}   # j -> [4 tiles]
        ct_all = {}   # j -> [4 tiles]

        # ---------- emission units ----------
        def x_load(j):
            xt = []
            for d in range(ND):
                x_t = px.tile([128, CH], bf16, name=f"x{d}", tag=f"x{d}")
                nc.sync.dma_start(
                    out=x_t, in_=xT[128 * d:128 * (d + 1), CH * j:CH * (j + 1)]
                )
                xt.append(x_t)
            return xt

        def proj_half(ps, w_sb, t, xt, half, kind):
            """4 of the 8 contraction steps of one projection m-tile."""
            for d in range(4 * half, 4 * half + 4):
                if kind == "v":
                    lhsT = xt[d][:, 128 * t:128 * (t + 1)]
                    rhs = w_sb[d]
                else:
                    lhsT = w_sb[d][:, 128 * t:128 * (t + 1)]
                    rhs = xt[d]
                nc.tensor.matmul(
                    ps, lhsT=lhsT, rhs=rhs,
                    start=(d == 0), stop=(d == ND - 1),
                )

        def qkv_units(j):
            """Generator of emission closures for chunk j's QKV projection."""
            xt = []

            def do_xload():
                xt.extend(x_load(j))
            yield do_xload

            qts = []
            qt_all[j] = qts
            for t in range(NMT):
                ps_box = []

                def qa(t=t, ps_box=ps_box):
                    ps = pp_mm.tile([128, CH], f32, name="psq", tag="mm")
                    ps_box.append(ps)
                    proj_half(ps, wq_sb, t, xt, 0, "q")
                def qb(t=t, ps_box=ps_box):
                    ps = ps_box[0]
                    proj_half(ps, wq_sb, t, xt, 1, "q")
                    q_t = pq.tile([128, CH], bf16, name=f"q{t}", tag=f"q{t}")
                    nc.vector.tensor_copy(out=q_t, in_=ps)
                    qts.append(q_t)
                yield qa
                yield qb
            for t in range(NMT):
                ps_box = []

                def ka(t=t, ps_box=ps_box):
                    ps = pp_mm.tile([128, CH], f32, name="psk", tag="mm")
                    ps_box.append(ps)
                    proj_half(ps, wk_sb, t, xt, 0, "k")
                def kb(t=t, ps_box=ps_box, j=j):
                    ps = ps_box[0]
                    proj_half(ps, wk_sb, t, xt, 1, "k")
                    k_t = pkv.tile(
                        [128, CH], bf16, name=f"k{t}_{j}", tag=f"k{t}_{j}"
                    )
                    nc.vector.tensor_copy(out=k_t, in_=ps)
                    kt_sb[t][j] = k_t
                yield ka
                yield kb
            for st in range(NMT):
                ps_box = []

                def va(st=st, ps_box=ps_box):
                    ps = pp_mm.tile([128, M], f32, name="psv", tag="mm")
                    ps_box.append(ps)
                    proj_half(ps, wv_sb, st, xt, 0, "v")
                def vb(st=st, ps_box=ps_box, j=j):
                    ps = ps_box[0]
                    proj_half(ps, wv_sb, st, xt, 1, "v")
                    g = vau[4 * j + st]
                    nc.vector.tensor_copy(
                        out=g.rearrange("p (h c) -> p h c", c=65)[:, :, 0:64],
                        in_=ps.rearrange("p (h c) -> p h c", c=64),
                    )
                yield va
                yield vb

        def outproj_units(j):
            """Generator of emission closures for chunk j's out-projection."""
            for nt in range(NNT):
                def og(nt=nt, j=j):
                    ct = ct_all[j]
                    ps = pp_mm.tile([128, CH], f32, name="pso", tag="mm")
                    for t in range(NMT):
                        nc.tensor.matmul(
                            ps,
                            lhsT=wo_sb[t][:, 128 * nt:128 * (nt + 1)],
                            rhs=ct[t],
                            start=(t == 0),
                            stop=(t == NMT - 1),
                        )
                    o_sb = po.tile([128, CH], f32, name="osb", tag="o")
                    nc.vector.tensor_copy(out=o_sb, in_=ps)
                    nc.sync.dma_start(
                        out=outT[128 * nt:128 * (nt + 1), CH * j:CH * (j + 1)],
                        in_=o_sb,
                    )
                yield og

        # ---------- chunk 0 QKV up front ----------
        for unit in qkv_units(0):
            unit()

        # ---------- main loop: attention(j) with interleaved fillers ----------
        for j in range(NCH):
            fillers = []
            if j + 1 < NCH:
                fillers.extend(qkv_units(j + 1))
            if j >= 1:
                fillers.extend(outproj_units(j - 1))
            nkt = 4 * (j + 1)
            n_units = NMT * (nkt + 1)
            n_fill = len(fillers)
            popped = 0
            ucount = 0

            qt = qt_all[j]
            ct = []
            ct_all[j] = ct
            den8 = psm.tile([HPC, CH], f32, name="den8", tag="den8")
            for t in range(NMT):
                av = [
                    pp_av.tile([65, CH], f32, name=f"av{h}", tag="av")
                    for h in range(2)
                ]
                us = {}
                for kt in range(nkt + 1):
                    if kt < nkt:
                        dd = kt - 4 * j      # diagonal index (>=0 on diag)
                        qoff = 128 * dd if dd >= 0 else 0
                        n = CH - qoff
                        ck, ks = kt // 4, (kt % 4) * 128
                        # both heads' scores in one 2-bank PSUM tile
                        sc = pp_sc.tile([128, 2 * CH], f32, name="sc", tag="sc")
                        for h in range(2):
                            pb = 64 * h
                            nc.tensor.matmul(
                                sc[:, CH * h:CH * h + n],
                                lhsT=kt_sb[t][ck][pb:pb + 64, ks:ks + 128],
                                rhs=qt[t][pb:pb + 64, qoff:CH],
                                start=True,
                                stop=True,
                                tile_position=(pb, 0),
                            )
                        u = pu.tile([128, 2 * CH], bf16, name="u", tag="u")
                        scv = sc.rearrange("p (h q) -> p h q", h=2)[:, :, 0:n]
                        uv = u.rearrange("p (h q) -> p h q", h=2)[:, :, 0:n]
                        nc.scalar.activation(out=uv, in_=scv, func=EXP, scale=0.125)
                        if dd >= 0:
                            # keep where q_rel >= k_partition (same mask, both)
                            nc.gpsimd.affine_select(
                                out=uv,
                                in_=uv,
                                compare_op=mybir.AluOpType.is_ge,
                                fill=0.0,
                                base=0,
                                channel_multiplier=-1,
                                pattern=[[0, 2], [1, n]],
                            )
                        us[kt] = (u, qoff, n)
                    if kt >= 1:
                        # attnV for the PREVIOUS kt (exp latency hidden by
                        # the scores matmul above and the filler below)
                        pkt = kt - 1
                        u_p, qoff_p, n_p = us.pop(pkt)
                        for h in range(2):
                            ha = 2 * t + h
                            nc.tensor.matmul(
                                av[h][:, qoff_p:CH],
                                lhsT=vau[pkt][:, 65 * ha:65 * ha + 65],
                                rhs=u_p[:, CH * h:CH * h + n_p],
                                start=(pkt == 0),
                                stop=(pkt == nkt - 1),
                            )
                    ucount += 1
                    while fillers and popped < ucount * n_fill // n_units:
                        fillers.pop(0)()
                        popped += 1

                # drain PSUM quickly: unnormalized C (bf16) + denom rows (f32)
                cn_t = pcn.tile([128, CH], bf16, name=f"cn{t}", tag=f"cn{t}")
                for h in range(2):
                    nc.vector.tensor_copy(
                        out=cn_t[64 * h:64 * (h + 1), :], in_=av[h][0:64, :]
                    )
                    dstage = psm.tile(
                        [1, CH], f32, name="dstage", tag="dstage", bufs=4
                    )
                    nc.vector.tensor_copy(out=dstage, in_=av[h][64:65, :])
                    nc.sync.dma_start(
                        out=den8[2 * t + h:2 * t + h + 1, :], in_=dstage
                    )
                ct.append(cn_t)  # placeholder; replaced after normalize

            # one batched reciprocal for all 8 heads, then normalize
            rec8 = psm.tile([HPC, CH], f32, name="rec8", tag="rec8")
            nc.vector.reciprocal(out=rec8, in_=den8)
            rec_d = pdram.tile([HPC, CH], f32, name="recd", tag="recd")
            nc.sync.dma_start(out=rec_d, in_=rec8)
            cn = list(ct)
            for t in range(NMT):
                c_t = pct.tile([128, CH], bf16, name=f"c{t}", tag=f"c{t}")
                bc = psm.tile([128, CH], f32, name="bc", tag="bc", bufs=4)
                for h in range(2):
                    nc.sync.dma_start(
                        out=bc[64 * h:64 * (h + 1), :],
                        in_=rec_d[2 * t + h:2 * t + h + 1, :].to_broadcast(
                            (64, CH)
                        ),
                    )
                for h in range(2):
                    nc.vector.tensor_mul(
                        c_t[64 * h:64 * (h + 1), :],
                        cn[t][64 * h:64 * (h + 1), :],
                        bc[64 * h:64 * (h + 1), :],
                    )
                ct[t] = c_t

            # leftover fillers for this round
            for f in fillers:
                f()

        # final chunk's out-projection
        for unit in outproj_units(NCH - 1):
            unit()


_PROG = None


def _build():
    global _PROG
    if _PROG is not None:
        return _PROG
    import concourse.bacc as bacc
    import concourse.mybir as mybir
    import concourse.tile as tile

    f32 = mybir.dt.float32
    bf16 = mybir.dt.bfloat16
    nc = bacc.Bacc(
        "TRN2", target_bir_lowering=False, debug=False, enable_asserts=False
    )
    xT = nc.dram_tensor("xT", [D, S], bf16, kind="ExternalInput").ap()
    wq = nc.dram_tensor("wq", [D, M], bf16, kind="ExternalInput").ap()
    wk = nc.dram_tensor("wk", [D, M], bf16, kind="ExternalInput").ap()
    wv = nc.dram_tensor("wv", [D, M], bf16, kind="ExternalInput").ap()
    wo = nc.dram_tensor("wo", [M, D], bf16, kind="ExternalInput").ap()
    ones8 = nc.dram_tensor("ones8", [128, HPC], bf16, kind="ExternalInput").ap()
    outT = nc.dram_tensor("outT", [D, S], f32, kind="ExternalOutput").ap()

    with tile.TileContext(nc) as tc:
        _emit(nc, tc, tile, mybir, (xT, wq, wk, wv, wo, ones8, outT))
    nc.compile()
    _PROG = nc
    return nc


def kernel(x, Wq, Wk, Wv, Wo, bo):
    global LAST_RESULT
    import os

    from concourse.bass_utils import run_bass_kernel_spmd

    x = np.asarray(x, dtype=np.float32)
    Wq = np.asarray(Wq, dtype=np.float32)
    Wk = np.asarray(Wk, dtype=np.float32)
    Wv = np.asarray(Wv, dtype=np.float32)
    Wo = np.asarray(Wo, dtype=np.float32)
    bo = np.asarray(bo, dtype=np.float32)

    nc = _build()

    import ml_dtypes

    bf = ml_dtypes.bfloat16
    in_maps = []
    for c in range(NCORE):
        b, g = c // 2, c % 2
        cols = slice(M * g, M * (g + 1))
        in_maps.append(
            {
                "xT": np.ascontiguousarray(x[b].T).astype(bf),
                "wq": np.ascontiguousarray(Wq[:, cols]).astype(bf),
                "wk": np.ascontiguousarray(Wk[:, cols]).astype(bf),
                "wv": np.ascontiguousarray(Wv[:, cols]).astype(bf),
                "wo": np.ascontiguousarray(Wo[cols, :]).astype(bf),
                "ones8": np.ones((128, HPC), dtype=bf),
            }
        )

    res = run_bass_kernel_spmd(
        nc,
        in_maps,
        list(range(NCORE)),
        trace=bool(os.environ.get("KERNEL_TRACE")),
        tmpdir=os.environ.get("KERNEL_TRACE_DIR") or None,
    )
    LAST_RESULT = res

    out = np.empty((B, S, D), dtype=np.float32)
    for b in range(B):
        acc = res.results[2 * b]["outT"] + res.results[2 * b + 1]["outT"]
        out[b] = acc.T + bo[None, :]
    return out


# revision 27
# speedup vs baseline: 1.8699x; 1.0219x over previous
"""Multi-head causal attention (B=4, S=2048, D=1024, H=16) on 8 TRN2 cores.

Sharding: core c -> batch c//2, head-group c%2 (8 heads, 512 of the 1024
QKV columns / Wo rows).  Each core runs a fused QKV->attention->out-proj
kernel on its shard; the host sums the two head-group partials per batch.

Per-core layout choices:
  - x is fed pre-transposed (xT [D, S]) so Q^T/K^T come out of the PE in
    [m, s] layout and V in natural [s, m] layout with no on-chip transposes.
  - scores are computed transposed (S^T [k, q]); softmax runs as
    exp (ScalarE, scale=1/8 fused, both heads of a pair in one op) ->
    causal mask (gpsimd affine_select, fill=0, diagonal tiles only,
    fully-masked q-ranges skipped entirely) -> attnV matmul with a
    ones-column appended to V (M=65) so the softmax denominator
    accumulates for free in PSUM row 64.
  - normalization: denominator rows are gathered into one [8, S-chunk]
    tile (via SBUF->SBUF DMA; compute engines cannot address partition
    bases other than 0/32/64/96), one batched DVE reciprocal, then a
    DRAM-bounced broadcast DMA and one DVE multiply into C^T [m, s].
  - out-proj emits out^T [n, s]; the host transposes back.
  - Tile builds STATIC per-engine instruction streams, so next-chunk QKV
    and previous-chunk out-proj matmuls are explicitly interleaved into
    the attention kt-loop to keep the PE dense (and HAM un-throttled)
    while ScalarE works on exp.
All matmul inputs are bf16 (1 cycle/row on the PE; fp32r is a 2-pass
format at ~2 cycles/row); accumulation stays fp32 in PSUM.
"""

import numpy as np

B, S, D = 4, 2048, 1024
H, DH = 16, 64
HPC = 8            # heads per core
M = HPC * DH       # 512: per-core qkv out dim / wo in dim
NCORE = 8
CH = 512           # q/s chunk size
NCH = S // CH      # 4
ND = D // 128      # 8  d-tiles (contraction for qkv proj)
NMT = M // 128     # 4  m-tiles (= head pairs)
NKT = S // 128     # 16 k-tiles
NNT = D // 128     # 8  n-tiles (out proj)

LAST_RESULT = None  # BassKernelResults of the most recent run (for test.py)


def _emit(nc, tc, tile, mybir, aps):
    import concourse.bass as bass  # noqa: F401

    f32 = mybir.dt.float32
    bf16 = mybir.dt.bfloat16
    EXP = mybir.ActivationFunctionType.Exp
    xT, wq, wk, wv, wo, ones8, outT = aps

    with (
        tc.tile_pool(name="w", bufs=1) as pw,
        tc.tile_pool(name="kv", bufs=1) as pkv,
        tc.tile_pool(name="qt", bufs=2) as pq,
        tc.tile_pool(name="ct", bufs=2) as pct,
        tc.tile_pool(name="x", bufs=2) as px,
        tc.tile_pool(name="u", bufs=6) as pu,
        tc.tile_pool(name="sm", bufs=2) as psm,
        tc.tile_pool(name="o", bufs=2) as po,
        tc.tile_pool(name="cn", bufs=2) as pcn,
        tc.tile_pool(name="dscratch", bufs=2, space="DRAM") as pdram,
        tc.tile_pool(name="ps_mm", bufs=2, space="PSUM") as pp_mm,
        tc.tile_pool(name="ps_sc", bufs=2, space="PSUM") as pp_sc,
        tc.tile_pool(name="ps_av", bufs=2, space="PSUM") as pp_av,
    ):
        # ---- weights: one big DMA per tensor, d-tiles along the free dim ----
        def load_folded(src_ap, nm, rows, cols):
            # src [rows*128? ...]: fold [(nd p), c] -> [p, (nd c)]
            ndt = rows // 128
            t = pw.tile([128, ndt * cols], bf16, name=nm, tag=nm)
            nc.sync.dma_start(
                out=t.rearrange("p (n c) -> p n c", c=cols),
                in_=src_ap.rearrange("(n p) c -> p n c", p=128),
            )
            return t

        wq_all = load_folded(wq, "wqa", D, M)
        wk_all = load_folded(wk, "wka", D, M)
        wv_all = load_folded(wv, "wva", D, M)
        wo_all = load_folded(wo, "woa", M, D)
        wq_sb = [wq_all[:, M * d:M * (d + 1)] for d in range(ND)]
        wk_sb = [wk_all[:, M * d:M * (d + 1)] for d in range(ND)]
        wv_sb = [wv_all[:, M * d:M * (d + 1)] for d in range(ND)]
        wo_sb = [wo_all[:, D * t:D * (t + 1)] for t in range(NMT)]

        # ---- V storage: [s, 8 heads x (64 V + 1 ones)] ----
        vau = []
        for st in range(NKT):
            v = pkv.tile([128, HPC * 65], bf16, name=f"vau{st}", tag=f"vau{st}")
            nc.sync.dma_start(
                out=v.rearrange("p (h c) -> p h c", c=65)[:, :, 64:65],
                in_=ones8.rearrange("p (h c) -> p h c", c=1),
            )
            vau.append(v)
        kt_sb = [[None] * NCH for _ in range(NMT)]
        qt_all = {}   # j -> [4 tiles]
        ct_all = {}   # j -> [4 tiles]

        # ---------- emission units ----------
        def x_load(j):
            xa = px.tile([128, ND * CH], bf16, name="xa", tag="xa")
            nc.sync.dma_start(
                out=xa.rearrange("p (n c) -> p n c", c=CH),
                in_=xT.rearrange("(n p) s -> p n s", p=128)[
                    :, :, CH * j:CH * (j + 1)
                ],
            )
            return [xa[:, CH * d:CH * (d + 1)] for d in range(ND)]

        def proj_half(ps, w_sb, t, xt, half, kind):
            """4 of the 8 contraction steps of one projection m-tile."""
            for d in range(4 * half, 4 * half + 4):
                if kind == "v":
                    lhsT = xt[d][:, 128 * t:128 * (t + 1)]
                    rhs = w_sb[d]
                else:
                    lhsT = w_sb[d][:, 128 * t:128 * (t + 1)]
                    rhs = xt[d]
                nc.tensor.matmul(
                    ps, lhsT=lhsT, rhs=rhs,
                    start=(d == 0), stop=(d == ND - 1),
                )

        def qkv_units(j):
            """Generator of emission closures for chunk j's QKV projection."""
            xt = []

            def do_xload():
                xt.extend(x_load(j))
            yield do_xload

            qts = []
            qt_all[j] = qts
            for t in range(NMT):
                ps_box = []

                def qa(t=t, ps_box=ps_box):
                    ps = pp_mm.tile([128, CH], f32, name="psq", tag="mm")
                    ps_box.append(ps)
                    proj_half(ps, wq_sb, t, xt, 0, "q")
                def qb(t=t, ps_box=ps_box):
                    ps = ps_box[0]
                    proj_half(ps, wq_sb, t, xt, 1, "q")
                    q_t = pq.tile([128, CH], bf16, name=f"q{t}", tag=f"q{t}")
                    nc.vector.tensor_copy(out=q_t, in_=ps)
                    qts.append(q_t)
                yield qa
                yield qb
            for t in range(NMT):
                ps_box = []

                def ka(t=t, ps_box=ps_box):
                    ps = pp_mm.tile([128, CH], f32, name="psk", tag="mm")
                    ps_box.append(ps)
                    proj_half(ps, wk_sb, t, xt, 0, "k")
                def kb(t=t, ps_box=ps_box, j=j):
                    ps = ps_box[0]
                    proj_half(ps, wk_sb, t, xt, 1, "k")
                    k_t = pkv.tile(
                        [128, CH], bf16, name=f"k{t}_{j}", tag=f"k{t}_{j}"
                    )
                    nc.vector.tensor_copy(out=k_t, in_=ps)
                    kt_sb[t][j] = k_t
                yield ka
                yield kb
            for st in range(NMT):
                ps_box = []

                def va(st=st, ps_box=ps_box):
                    ps = pp_mm.tile([128, M], f32, name="psv", tag="mm")
                    ps_box.append(ps)
                    proj_half(ps, wv_sb, st, xt, 0, "v")
                def vb(st=st, ps_box=ps_box, j=j):
                    ps = ps_box[0]
                    proj_half(ps, wv_sb, st, xt, 1, "v")
                    g = vau[4 * j + st]
                    nc.vector.tensor_copy(
                        out=g.rearrange("p (h c) -> p h c", c=65)[:, :, 0:64],
                        in_=ps.rearrange("p (h c) -> p h c", c=64),
                    )
                yield va
                yield vb

        def outproj_units(j):
            """Generator of emission closures for chunk j's out-projection."""
            for nt in range(NNT):
                def og(nt=nt, j=j):
                    ct = ct_all[j]
                    ps = pp_mm.tile([128, CH], f32, name="pso", tag="mm")
                    for t in range(NMT):
                        nc.tensor.matmul(
                            ps,
                            lhsT=wo_sb[t][:, 128 * nt:128 * (nt + 1)],
                            rhs=ct[t],
                            start=(t == 0),
                            stop=(t == NMT - 1),
                        )
                    o_sb = po.tile([128, CH], f32, name="osb", tag="o")
                    nc.vector.tensor_copy(out=o_sb, in_=ps)
                    nc.sync.dma_start(
                        out=outT[128 * nt:128 * (nt + 1), CH * j:CH * (j + 1)],
                        in_=o_sb,
                    )
                yield og

        # ---------- chunk 0 QKV up front ----------
        for unit in qkv_units(0):
            unit()

        # ---------- main loop: attention(j) with interleaved fillers ----------
        for j in range(NCH):
            fillers = []
            if j + 1 < NCH:
                fillers.extend(qkv_units(j + 1))
            if j >= 1:
                fillers.extend(outproj_units(j - 1))
            nkt = 4 * (j + 1)
            n_units = NMT * (nkt + 1)
            n_fill = len(fillers)
            popped = 0
            ucount = 0

            qt = qt_all[j]
            ct = []
            ct_all[j] = ct
            den8 = psm.tile([HPC, CH], f32, name="den8", tag="den8")
            for t in range(NMT):
                av = [
                    pp_av.tile([65, CH], f32, name=f"av{h}", tag="av")
                    for h in range(2)
                ]
                us = {}
                for kt in range(nkt + 1):
                    if kt < nkt:
                        dd = kt - 4 * j      # diagonal index (>=0 on diag)
                        qoff = 128 * dd if dd >= 0 else 0
                        n = CH - qoff
                        ck, ks = kt // 4, (kt % 4) * 128
                        # both heads' scores in one 2-bank PSUM tile
                        sc = pp_sc.tile([128, 2 * CH], f32, name="sc", tag="sc")
                        for h in range(2):
                            pb = 64 * h
                            nc.tensor.matmul(
                                sc[:, CH * h:CH * h + n],
                                lhsT=kt_sb[t][ck][pb:pb + 64, ks:ks + 128],
                                rhs=qt[t][pb:pb + 64, qoff:CH],
                                start=True,
                                stop=True,
                                tile_position=(pb, 0),
                            )
                        u = pu.tile([128, 2 * CH], bf16, name="u", tag="u")
                        scv = sc.rearrange("p (h q) -> p h q", h=2)[:, :, 0:n]
                        uv = u.rearrange("p (h q) -> p h q", h=2)[:, :, 0:n]
                        nc.scalar.activation(out=uv, in_=scv, func=EXP, scale=0.125)
                        if dd >= 0:
                            # keep where q_rel >= k_partition (same mask, both)
                            nc.gpsimd.affine_select(
                                out=uv,
                                in_=uv,
                                compare_op=mybir.AluOpType.is_ge,
                                fill=0.0,
                                base=0,
                                channel_multiplier=-1,
                                pattern=[[0, 2], [1, n]],
                            )
                        us[kt] = (u, qoff, n)
                    if kt >= 1:
                        # attnV for the PREVIOUS kt (exp latency hidden by
                        # the scores matmul above and the filler below)
                        pkt = kt - 1
                        u_p, qoff_p, n_p = us.pop(pkt)
                        for h in range(2):
                            ha = 2 * t + h
                            nc.tensor.matmul(
                                av[h][:, qoff_p:CH],
                                lhsT=vau[pkt][:, 65 * ha:65 * ha + 65],
                                rhs=u_p[:, CH * h:CH * h + n_p],
                                start=(pkt == 0),
                                stop=(pkt == nkt - 1),
                            )
                    ucount += 1
                    while fillers and popped < ucount * n_fill // n_units:
                        fillers.pop(0)()
                        popped += 1

                # drain PSUM quickly: unnormalized C (bf16) + denom rows (f32)
                cn_t = pcn.tile([128, CH], bf16, name=f"cn{t}", tag=f"cn{t}")
                for h in range(2):
                    nc.vector.tensor_copy(
                        out=cn_t[64 * h:64 * (h + 1), :], in_=av[h][0:64, :]
                    )
                    dstage = psm.tile(
                        [1, CH], f32, name="dstage", tag="dstage", bufs=4
                    )
                    nc.vector.tensor_copy(out=dstage, in_=av[h][64:65, :])
                    nc.sync.dma_start(
                        out=den8[2 * t + h:2 * t + h + 1, :], in_=dstage
                    )
                ct.append(cn_t)  # placeholder; replaced after normalize

            # one batched reciprocal for all 8 heads, then normalize
            rec8 = psm.tile([HPC, CH], f32, name="rec8", tag="rec8")
            nc.vector.reciprocal(out=rec8, in_=den8)
            rec_d = pdram.tile([HPC, CH], f32, name="recd", tag="recd")
            nc.sync.dma_start(out=rec_d, in_=rec8)
            cn = list(ct)
            for t in range(NMT):
                c_t = pct.tile([128, CH], bf16, name=f"c{t}", tag=f"c{t}")
                bc = psm.tile([128, CH], f32, name="bc", tag="bc", bufs=4)
                for h in range(2):
                    nc.sync.dma_start(
                        out=bc[64 * h:64 * (h + 1), :],
                        in_=rec_d[2 * t + h:2 * t + h + 1, :].to_broadcast(
                            (64, CH)
                        ),
                    )
                for h in range(2):
                    nc.vector.tensor_mul(
                        c_t[64 * h:64 * (h + 1), :],
                        cn[t][64 * h:64 * (h + 1), :],
                        bc[64 * h:64 * (h + 1), :],
                    )
                ct[t] = c_t

            # leftover fillers for this round
            for f in fillers:
                f()

        # final chunk's out-projection
        for unit in outproj_units(NCH - 1):
            unit()


_PROG = None


def _build():
    global _PROG
    if _PROG is not None:
        return _PROG
    import concourse.bacc as bacc
    import concourse.mybir as mybir
    import concourse.tile as tile

    f32 = mybir.dt.float32
    bf16 = mybir.dt.bfloat16
    nc = bacc.Bacc(
        "TRN2", target_bir_lowering=False, debug=False, enable_asserts=False
    )
    xT = nc.dram_tensor("xT", [D, S], bf16, kind="ExternalInput").ap()
    wq = nc.dram_tensor("wq", [D, M], bf16, kind="ExternalInput").ap()
    wk = nc.dram_tensor("wk", [D, M], bf16, kind="ExternalInput").ap()
    wv = nc.dram_tensor("wv", [D, M], bf16, kind="ExternalInput").ap()
    wo = nc.dram_tensor("wo", [M, D], bf16, kind="ExternalInput").ap()
    ones8 = nc.dram_tensor("ones8", [128, HPC], bf16, kind="ExternalInput").ap()
    outT = nc.dram_tensor("outT", [D, S], f32, kind="ExternalOutput").ap()

    with tile.TileContext(nc) as tc:
        _emit(nc, tc, tile, mybir, (xT, wq, wk, wv, wo, ones8, outT))
    nc.compile()
    _PROG = nc
    return nc


def kernel(x, Wq, Wk, Wv, Wo, bo):
    global LAST_RESULT
    import os

    from concourse.bass_utils import run_bass_kernel_spmd

    x = np.asarray(x, dtype=np.float32)
    Wq = np.asarray(Wq, dtype=np.float32)
    Wk = np.asarray(Wk, dtype=np.float32)
    Wv = np.asarray(Wv, dtype=np.float32)
    Wo = np.asarray(Wo, dtype=np.float32)
    bo = np.asarray(bo, dtype=np.float32)

    nc = _build()

    import ml_dtypes

    bf = ml_dtypes.bfloat16
    in_maps = []
    for c in range(NCORE):
        b, g = c // 2, c % 2
        cols = slice(M * g, M * (g + 1))
        in_maps.append(
            {
                "xT": np.ascontiguousarray(x[b].T).astype(bf),
                "wq": np.ascontiguousarray(Wq[:, cols]).astype(bf),
                "wk": np.ascontiguousarray(Wk[:, cols]).astype(bf),
                "wv": np.ascontiguousarray(Wv[:, cols]).astype(bf),
                "wo": np.ascontiguousarray(Wo[cols, :]).astype(bf),
                "ones8": np.ones((128, HPC), dtype=bf),
            }
        )

    res = run_bass_kernel_spmd(
        nc,
        in_maps,
        list(range(NCORE)),
        trace=bool(os.environ.get("KERNEL_TRACE")),
        tmpdir=os.environ.get("KERNEL_TRACE_DIR") or None,
    )
    LAST_RESULT = res

    out = np.empty((B, S, D), dtype=np.float32)
    for b in range(B):
        acc = res.results[2 * b]["outT"] + res.results[2 * b + 1]["outT"]
        out[b] = acc.T + bo[None, :]
    return out


# revision 28
# speedup vs baseline: 1.8753x; 1.0029x over previous
"""Multi-head causal attention (B=4, S=2048, D=1024, H=16) on 8 TRN2 cores.

Sharding: core c -> batch c//2, head-group c%2 (8 heads, 512 of the 1024
QKV columns / Wo rows).  Each core runs a fused QKV->attention->out-proj
kernel on its shard; the host sums the two head-group partials per batch.

Per-core layout choices:
  - x is fed pre-transposed (xT [D, S]) so Q^T/K^T come out of the PE in
    [m, s] layout and V in natural [s, m] layout with no on-chip transposes.
  - scores are computed transposed (S^T [k, q]); softmax runs as
    exp (ScalarE, scale=1/8 fused, both heads of a pair in one op) ->
    causal mask (gpsimd affine_select, fill=0, diagonal tiles only,
    fully-masked q-ranges skipped entirely) -> attnV matmul with a
    ones-column appended to V (M=65) so the softmax denominator
    accumulates for free in PSUM row 64.
  - normalization: denominator rows are gathered into one [8, S-chunk]
    tile (via SBUF->SBUF DMA; compute engines cannot address partition
    bases other than 0/32/64/96), one batched DVE reciprocal, then a
    DRAM-bounced broadcast DMA and one DVE multiply into C^T [m, s].
  - out-proj emits out^T [n, s]; the host transposes back.
  - Tile builds STATIC per-engine instruction streams, so next-chunk QKV
    and previous-chunk out-proj matmuls are explicitly interleaved into
    the attention kt-loop to keep the PE dense (and HAM un-throttled)
    while ScalarE works on exp.
All matmul inputs are bf16 (1 cycle/row on the PE; fp32r is a 2-pass
format at ~2 cycles/row); accumulation stays fp32 in PSUM.
"""

import numpy as np

B, S, D = 4, 2048, 1024
H, DH = 16, 64
HPC = 8            # heads per core
M = HPC * DH       # 512: per-core qkv out dim / wo in dim
NCORE = 8
CH = 512           # q/s chunk size
NCH = S // CH      # 4
ND = D // 128      # 8  d-tiles (contraction for qkv proj)
NMT = M // 128     # 4  m-tiles (= head pairs)
NKT = S // 128     # 16 k-tiles
NNT = D // 128     # 8  n-tiles (out proj)

LAST_RESULT = None  # BassKernelResults of the most recent run (for test.py)


def _emit(nc, tc, tile, mybir, aps):
    import concourse.bass as bass  # noqa: F401

    f32 = mybir.dt.float32
    bf16 = mybir.dt.bfloat16
    EXP = mybir.ActivationFunctionType.Exp
    xT, wq, wk, wv, wo, ones8, outT = aps

    with (
        tc.tile_pool(name="w", bufs=1) as pw,
        tc.tile_pool(name="kv", bufs=1) as pkv,
        tc.tile_pool(name="qt", bufs=2) as pq,
        tc.tile_pool(name="ct", bufs=2) as pct,
        tc.tile_pool(name="x", bufs=2) as px,
        tc.tile_pool(name="u", bufs=6) as pu,
        tc.tile_pool(name="sm", bufs=2) as psm,
        tc.tile_pool(name="o", bufs=2) as po,
        tc.tile_pool(name="cn", bufs=2) as pcn,
        tc.tile_pool(name="dscratch", bufs=2, space="DRAM") as pdram,
        tc.tile_pool(name="ps_mm", bufs=2, space="PSUM") as pp_mm,
        tc.tile_pool(name="ps_sc", bufs=2, space="PSUM") as pp_sc,
        tc.tile_pool(name="ps_av", bufs=2, space="PSUM") as pp_av,
    ):
        # ---- weights: host pre-folded to SBUF layout; contiguous DMAs ----
        def load_flat(src_ap, nm, cols):
            t = pw.tile([128, cols], bf16, name=nm, tag=nm)
            nc.sync.dma_start(out=t, in_=src_ap)
            return t

        wq_all = load_flat(wq, "wqa", ND * M)
        wk_all = load_flat(wk, "wka", ND * M)
        wv_all = load_flat(wv, "wva", ND * M)
        wo_all = load_flat(wo, "woa", NMT * D)
        wq_sb = [wq_all[:, M * d:M * (d + 1)] for d in range(ND)]
        wk_sb = [wk_all[:, M * d:M * (d + 1)] for d in range(ND)]
        wv_sb = [wv_all[:, M * d:M * (d + 1)] for d in range(ND)]
        wo_sb = [wo_all[:, D * t:D * (t + 1)] for t in range(NMT)]

        # ---- V storage: [s, 8 heads x (64 V + 1 ones)] ----
        vau = []
        for st in range(NKT):
            v = pkv.tile([128, HPC * 65], bf16, name=f"vau{st}", tag=f"vau{st}")
            nc.sync.dma_start(
                out=v.rearrange("p (h c) -> p h c", c=65)[:, :, 64:65],
                in_=ones8.rearrange("p (h c) -> p h c", c=1),
            )
            vau.append(v)
        kt_sb = [[None] * NCH for _ in range(NMT)]
        qt_all = {}   # j -> [4 tiles]
        ct_all = {}   # j -> [4 tiles]

        # ---------- emission units ----------
        def x_load(j):
            xa = px.tile([128, ND * CH], bf16, name="xa", tag="xa")
            nc.sync.dma_start(
                out=xa, in_=xT[:, ND * CH * j:ND * CH * (j + 1)]
            )
            return [xa[:, CH * d:CH * (d + 1)] for d in range(ND)]

        def proj_half(ps, w_sb, t, xt, half, kind):
            """4 of the 8 contraction steps of one projection m-tile."""
            for d in range(4 * half, 4 * half + 4):
                if kind == "v":
                    lhsT = xt[d][:, 128 * t:128 * (t + 1)]
                    rhs = w_sb[d]
                else:
                    lhsT = w_sb[d][:, 128 * t:128 * (t + 1)]
                    rhs = xt[d]
                nc.tensor.matmul(
                    ps, lhsT=lhsT, rhs=rhs,
                    start=(d == 0), stop=(d == ND - 1),
                )

        def qkv_units(j):
            """Generator of emission closures for chunk j's QKV projection."""
            xt = []

            def do_xload():
                xt.extend(x_load(j))
            yield do_xload

            qts = []
            qt_all[j] = qts
            for t in range(NMT):
                ps_box = []

                def qa(t=t, ps_box=ps_box):
                    ps = pp_mm.tile([128, CH], f32, name="psq", tag="mm")
                    ps_box.append(ps)
                    proj_half(ps, wq_sb, t, xt, 0, "q")
                def qb(t=t, ps_box=ps_box):
                    ps = ps_box[0]
                    proj_half(ps, wq_sb, t, xt, 1, "q")
                    q_t = pq.tile([128, CH], bf16, name=f"q{t}", tag=f"q{t}")
                    nc.vector.tensor_copy(out=q_t, in_=ps)
                    qts.append(q_t)
                yield qa
                yield qb
            for t in range(NMT):
                ps_box = []

                def ka(t=t, ps_box=ps_box):
                    ps = pp_mm.tile([128, CH], f32, name="psk", tag="mm")
                    ps_box.append(ps)
                    proj_half(ps, wk_sb, t, xt, 0, "k")
                def kb(t=t, ps_box=ps_box, j=j):
                    ps = ps_box[0]
                    proj_half(ps, wk_sb, t, xt, 1, "k")
                    k_t = pkv.tile(
                        [128, CH], bf16, name=f"k{t}_{j}", tag=f"k{t}_{j}"
                    )
                    nc.vector.tensor_copy(out=k_t, in_=ps)
                    kt_sb[t][j] = k_t
                yield ka
                yield kb
            for st in range(NMT):
                ps_box = []

                def va(st=st, ps_box=ps_box):
                    ps = pp_mm.tile([128, M], f32, name="psv", tag="mm")
                    ps_box.append(ps)
                    proj_half(ps, wv_sb, st, xt, 0, "v")
                def vb(st=st, ps_box=ps_box, j=j):
                    ps = ps_box[0]
                    proj_half(ps, wv_sb, st, xt, 1, "v")
                    g = vau[4 * j + st]
                    nc.vector.tensor_copy(
                        out=g.rearrange("p (h c) -> p h c", c=65)[:, :, 0:64],
                        in_=ps.rearrange("p (h c) -> p h c", c=64),
                    )
                yield va
                yield vb

        def outproj_units(j):
            """Generator of emission closures for chunk j's out-projection."""
            for nt in range(NNT):
                def og(nt=nt, j=j):
                    ct = ct_all[j]
                    ps = pp_mm.tile([128, CH], f32, name="pso", tag="mm")
                    for t in range(NMT):
                        nc.tensor.matmul(
                            ps,
                            lhsT=wo_sb[t][:, 128 * nt:128 * (nt + 1)],
                            rhs=ct[t],
                            start=(t == 0),
                            stop=(t == NMT - 1),
                        )
                    o_sb = po.tile([128, CH], f32, name="osb", tag="o")
                    nc.vector.tensor_copy(out=o_sb, in_=ps)
                    nc.sync.dma_start(
                        out=outT[128 * nt:128 * (nt + 1), CH * j:CH * (j + 1)],
                        in_=o_sb,
                    )
                yield og

        # ---------- chunk 0 QKV up front ----------
        u0 = list(qkv_units(0))
        u0[0]()  # x chunk-0 DMA first
        for unit in u0[1:]:
            unit()

        # ---------- main loop: attention(j) with interleaved fillers ----------
        for j in range(NCH):
            fillers = []
            if j + 1 < NCH:
                fillers.extend(qkv_units(j + 1))
            if j >= 1:
                fillers.extend(outproj_units(j - 1))
            nkt = 4 * (j + 1)
            n_units = NMT * (nkt + 1)
            n_fill = len(fillers)
            popped = 0
            ucount = 0

            qt = qt_all[j]
            ct = []
            ct_all[j] = ct
            den8 = psm.tile([HPC, CH], f32, name="den8", tag="den8")
            for t in range(NMT):
                av = [
                    pp_av.tile([65, CH], f32, name=f"av{h}", tag="av")
                    for h in range(2)
                ]
                us = {}
                for kt in range(nkt + 1):
                    if kt < nkt:
                        dd = kt - 4 * j      # diagonal index (>=0 on diag)
                        qoff = 128 * dd if dd >= 0 else 0
                        n = CH - qoff
                        ck, ks = kt // 4, (kt % 4) * 128
                        # both heads' scores in one 2-bank PSUM tile
                        sc = pp_sc.tile([128, 2 * CH], f32, name="sc", tag="sc")
                        for h in range(2):
                            pb = 64 * h
                            nc.tensor.matmul(
                                sc[:, CH * h:CH * h + n],
                                lhsT=kt_sb[t][ck][pb:pb + 64, ks:ks + 128],
                                rhs=qt[t][pb:pb + 64, qoff:CH],
                                start=True,
                                stop=True,
                                tile_position=(pb, 0),
                            )
                        u = pu.tile([128, 2 * CH], bf16, name="u", tag="u")
                        scv = sc.rearrange("p (h q) -> p h q", h=2)[:, :, 0:n]
                        uv = u.rearrange("p (h q) -> p h q", h=2)[:, :, 0:n]
                        nc.scalar.activation(out=uv, in_=scv, func=EXP, scale=0.125)
                        if dd >= 0:
                            # keep where q_rel >= k_partition (same mask, both)
                            nc.gpsimd.affine_select(
                                out=uv,
                                in_=uv,
                                compare_op=mybir.AluOpType.is_ge,
                                fill=0.0,
                                base=0,
                                channel_multiplier=-1,
                                pattern=[[0, 2], [1, n]],
                            )
                        us[kt] = (u, qoff, n)
                    if kt >= 1:
                        # attnV for the PREVIOUS kt (exp latency hidden by
                        # the scores matmul above and the filler below)
                        pkt = kt - 1
                        u_p, qoff_p, n_p = us.pop(pkt)
                        for h in range(2):
                            ha = 2 * t + h
                            nc.tensor.matmul(
                                av[h][:, qoff_p:CH],
                                lhsT=vau[pkt][:, 65 * ha:65 * ha + 65],
                                rhs=u_p[:, CH * h:CH * h + n_p],
                                start=(pkt == 0),
                                stop=(pkt == nkt - 1),
                            )
                    ucount += 1
                    while fillers and popped < ucount * n_fill // n_units:
                        fillers.pop(0)()
                        popped += 1

                # drain PSUM quickly: unnormalized C (bf16) + denom rows (f32)
                cn_t = pcn.tile([128, CH], bf16, name=f"cn{t}", tag=f"cn{t}")
                for h in range(2):
                    nc.vector.tensor_copy(
                        out=cn_t[64 * h:64 * (h + 1), :], in_=av[h][0:64, :]
                    )
                    dstage = psm.tile(
                        [1, CH], f32, name="dstage", tag="dstage", bufs=4
                    )
                    nc.vector.tensor_copy(out=dstage, in_=av[h][64:65, :])
                    nc.sync.dma_start(
                        out=den8[2 * t + h:2 * t + h + 1, :], in_=dstage
                    )
                ct.append(cn_t)  # placeholder; replaced after normalize

            # one batched reciprocal for all 8 heads, then normalize
            rec8 = psm.tile([HPC, CH], f32, name="rec8", tag="rec8")
            nc.vector.reciprocal(out=rec8, in_=den8)
            rec_d = pdram.tile([HPC, CH], f32, name="recd", tag="recd")
            nc.sync.dma_start(out=rec_d, in_=rec8)
            cn = list(ct)
            for t in range(NMT):
                c_t = pct.tile([128, CH], bf16, name=f"c{t}", tag=f"c{t}")
                bc = psm.tile([128, CH], f32, name="bc", tag="bc", bufs=4)
                for h in range(2):
                    nc.sync.dma_start(
                        out=bc[64 * h:64 * (h + 1), :],
                        in_=rec_d[2 * t + h:2 * t + h + 1, :].to_broadcast(
                            (64, CH)
                        ),
                    )
                for h in range(2):
                    nc.vector.tensor_mul(
                        c_t[64 * h:64 * (h + 1), :],
                        cn[t][64 * h:64 * (h + 1), :],
                        bc[64 * h:64 * (h + 1), :],
                    )
                ct[t] = c_t

            # leftover fillers for this round
            for f in fillers:
                f()

        # final chunk's out-projection
        for unit in outproj_units(NCH - 1):
            unit()


_PROG = None


def _build():
    global _PROG
    if _PROG is not None:
        return _PROG
    import concourse.bacc as bacc
    import concourse.mybir as mybir
    import concourse.tile as tile

    f32 = mybir.dt.float32
    bf16 = mybir.dt.bfloat16
    nc = bacc.Bacc(
        "TRN2", target_bir_lowering=False, debug=False, enable_asserts=False
    )
    xT = nc.dram_tensor("xT", [128, NCH * ND * CH], bf16, kind="ExternalInput").ap()
    wq = nc.dram_tensor("wq", [128, ND * M], bf16, kind="ExternalInput").ap()
    wk = nc.dram_tensor("wk", [128, ND * M], bf16, kind="ExternalInput").ap()
    wv = nc.dram_tensor("wv", [128, ND * M], bf16, kind="ExternalInput").ap()
    wo = nc.dram_tensor("wo", [128, NMT * D], bf16, kind="ExternalInput").ap()
    ones8 = nc.dram_tensor("ones8", [128, HPC], bf16, kind="ExternalInput").ap()
    outT = nc.dram_tensor("outT", [D, S], f32, kind="ExternalOutput").ap()

    with tile.TileContext(nc) as tc:
        _emit(nc, tc, tile, mybir, (xT, wq, wk, wv, wo, ones8, outT))
    nc.compile()
    _PROG = nc
    return nc


def kernel(x, Wq, Wk, Wv, Wo, bo):
    global LAST_RESULT
    import os

    from concourse.bass_utils import run_bass_kernel_spmd

    x = np.asarray(x, dtype=np.float32)
    Wq = np.asarray(Wq, dtype=np.float32)
    Wk = np.asarray(Wk, dtype=np.float32)
    Wv = np.asarray(Wv, dtype=np.float32)
    Wo = np.asarray(Wo, dtype=np.float32)
    bo = np.asarray(bo, dtype=np.float32)

    nc = _build()

    import ml_dtypes

    bf = ml_dtypes.bfloat16

    def fold_w(w):
        # [(nd p), c] -> [p, (nd c)]
        ndt = w.shape[0] // 128
        return np.ascontiguousarray(
            w.reshape(ndt, 128, w.shape[1]).transpose(1, 0, 2).reshape(128, -1)
        ).astype(bf)

    in_maps = []
    for c in range(NCORE):
        b, g = c // 2, c % 2
        cols = slice(M * g, M * (g + 1))
        xt = x[b].T  # [D, S]
        # [p, (j d s)]: xf[p, j*ND*CH + d*CH + s] = xT[128d+p, CH*j+s]
        xf = (
            xt.reshape(ND, 128, NCH, CH)
            .transpose(1, 2, 0, 3)
            .reshape(128, NCH * ND * CH)
        )
        in_maps.append(
            {
                "xT": np.ascontiguousarray(xf).astype(bf),
                "wq": fold_w(Wq[:, cols]),
                "wk": fold_w(Wk[:, cols]),
                "wv": fold_w(Wv[:, cols]),
                "wo": fold_w(Wo[cols, :]),
                "ones8": np.ones((128, HPC), dtype=bf),
            }
        )

    res = run_bass_kernel_spmd(
        nc,
        in_maps,
        list(range(NCORE)),
        trace=bool(os.environ.get("KERNEL_TRACE")),
        tmpdir=os.environ.get("KERNEL_TRACE_DIR") or None,
    )
    LAST_RESULT = res

    out = np.empty((B, S, D), dtype=np.float32)
    for b in range(B):
        acc = res.results[2 * b]["outT"] + res.results[2 * b + 1]["outT"]
        out[b] = acc.T + bo[None, :]
    return out
